# revision 15
# baseline (speedup 1.0000x reference)
"""Trainium2 Bass kernel for the DiffusionProcess problem (v4).

Strategy (hardcoded for B=2048, R=512, Z=256, H=512, T=16, 8 cores):
  - Data parallel: batch sharded 8 x 256, weights replicated.
  - Feature-major layout: activations [feature, batch]; matmuls
    out[M,N] = W[K,M].T @ x[K,N] with K,M tiles of 128, N = 256.
  - All matmul operands bf16 (1 row/cycle on the PE vs ~1.3 for f32r);
    PSUM accumulation stays f32.
  - Host-side weight folding (pure weight preprocessing):
      W_eff = dt * (Wh @ Wh @ Wo)            (no relu between them)
      const = (bh@Wh + bh)@Wo + bo
      c[t]  = temb_t @ W0 + b0               (per-step ACT bias columns)
      eps'[t] = sqrt_dt*eps[t] + dt*const    (folded into noise stream)
  - THE SCAN STATE LIVES IN PSUM: ps_d[mz] is initialized once with
    I@z0 (start=True) and then only accumulated into (start=False) for
    all 16 steps: each step adds I@eps'[t] plus sum_k a[k]@W_eff[k,mz].
    The f32 PSUM accumulation means the state is never rounded; the
    only per-step evacuation is a bf16 cast (DVE) that feeds the next
    step's matmuls and the zs output DMA.
  - Per-step device work:
      ps_a[m] = I@rw[m] + z_bf16@Wz[:,m]     (rw = r@Wr, on-device once)
      a[m]    = relu(ps_a[m] + c[m,t])       (ACT m=0,2 / DVE m=1,3)
      ps_d[mz]+= I@eps'[t,mz] + sum_k a[k] @ W_eff[k,mz]
      z_bf16  = cast(ps_d)                   (DVE, matmul rhs + output)
  - mus is NOT computed on device: mu = z - sqrt_dt*eps exactly, so the
    host reconstructs it from zs and the noise input. sigmas are
    constant. Only zs streams out (bf16), one DMA per step on the sync
    queue (whose HWDGE ring is otherwise idle during the scan).
  - Step-boundary retiming: stage-S is z-tile-major so cast(z'[0]) lands
    while S still works on z'[1]; the next step opens with 4 rw-inject
    matmuls (step-invariant operands) to cover the DVE tail latency.
  - HAM warmup: dummy matmuls keep the PE busy during the preamble DMA
    phase so the scan starts at the full 2.4 GHz clock; preamble DMAs
    are ordered so the rw dependencies (wrb, rtb) land first.
"""

import sys

if "/opt/trn_rl_repo" not in sys.path:
    sys.path.insert(0, "/opt/trn_rl_repo")

import numpy as np
import ml_dtypes

B, R, Z, H = 2048, 512, 256, 512
ZR = Z + R
T = 16
NC = 8
BS = B // NC          # 256 batch per core
DT = 1.0 / T
SQDT = DT ** 0.5
P = 128
KZ = Z // P           # 2
KR = R // P           # 4
KH = H // P           # 4
MH = H // P           # 4
MZ = Z // P           # 2
ZB = KZ * BS          # 512 = one step of z, feature-major columns
NCH = T // 2          # 8 two-step chunks for the eps input stream
BF16 = ml_dtypes.bfloat16

N_WARMUP = 26         # PE warmup matmuls during preamble DMA

_CACHE = {}


def _build():
    import concourse.bacc as bacc
    import concourse.tile as tile
    from concourse import mybir
    from concourse.tile_rust import add_dep_helper

    F32 = mybir.dt.float32
    BF = mybir.dt.bfloat16
    AF = mybir.ActivationFunctionType
    OP = mybir.AluOpType

    nc = bacc.Bacc("TRN2", target_bir_lowering=False, debug=False,
                   num_devices=NC)

    d_id = nc.dram_tensor("identb", [P, P], BF, kind="ExternalInput").ap()
    # pkb packs wzb [128, KZ*H] and z0b [128, ZB] into one DMA
    d_pk = nc.dram_tensor("pkb", [P, KZ * H + ZB], BF,
                          kind="ExternalInput").ap()
    d_wr = nc.dram_tensor("wrb", [P, KR * H], BF, kind="ExternalInput").ap()
    d_we = nc.dram_tensor("weffb", [P, KH * Z], BF,
                          kind="ExternalInput").ap()
    d_rt = nc.dram_tensor("rtb", [P, KR * BS], BF,
                          kind="ExternalInput").ap()
    d_ct = nc.dram_tensor("ctabb", [P, MH * T], F32,
                          kind="ExternalInput").ap()
    d_eps = nc.dram_tensor("epsb", [NCH, P, 2 * ZB], BF,
                           kind="ExternalInput").ap()
    d_zs = nc.dram_tensor("zsb", [T, P, ZB], BF,
                          kind="ExternalOutput").ap()

    with tile.TileContext(nc) as tc:
        with tc.tile_pool(name="w", bufs=1) as wp, \
             tc.tile_pool(name="act", bufs=2) as ap_, \
             tc.tile_pool(name="st", bufs=2) as sp, \
             tc.tile_pool(name="ps", bufs=1, space="PSUM") as pp:

            # ---- preamble DMA; rw deps (rtb, wrb) first on each queue ----
            identb = wp.tile([P, P], BF, tag="identb", name="identb")
            nc.sync.dma_start(identb[:], d_id[:])
            warmt = identb

            rtb = wp.tile([P, KR * BS], BF, tag="rtb", name="rtb")
            nc.sync.dma_start(rtb[:], d_rt[:])
            wrb = wp.tile([P, KR * H], BF, tag="wrb", name="wrb")
            nc.scalar.dma_start(wrb[:, 2 * H:], d_wr[:, 2 * H:])
            nc.sync.dma_start(wrb[:, :2 * H], d_wr[:, :2 * H])

            pkb = wp.tile([P, KZ * H + ZB], BF, tag="pkb", name="pkb")
            nc.gpsimd.dma_start(pkb[:], d_pk[:])
            wzb = pkb[:, :KZ * H]
            z0b = pkb[:, KZ * H:]

            ctab = wp.tile([P, MH * T], F32, tag="ctab", name="ctab")
            nc.scalar.dma_start(ctab[:], d_ct[:])
            weffb = wp.tile([P, KH * Z], BF, tag="weffb", name="weffb")
            nc.gpsimd.dma_start(weffb[:], d_we[:])

            # one-time ucode/table loads, off the critical path
            dumv = wp.tile([P, 8], F32, tag="dum", name="dum")
            nc.scalar.activation(dumv[:, 0:1], identb[:, 0:1], AF.Relu)
            nc.vector.tensor_copy(dumv[:, 1:2], identb[:, 1:2])

            # eps prefetch ring: chunk 0 early on the scalar HWDGE queue,
            # the rest stream on the gpsimd SWDGE queue
            eps_tiles = {}

            def eps_prefetch(c, eng=None):
                if c < NCH:
                    et = sp.tile([P, 2 * ZB], BF, tag="eps",
                                 name=f"eps_{c}", bufs=5)
                    (eng or nc.gpsimd).dma_start(et[:], d_eps[c])
                    eps_tiles[c] = et

            eps_prefetch(0, nc.scalar)
            for c in range(1, 4):
                eps_prefetch(c)

            def wz(k, m):
                return wzb[:, k * H + m * P: k * H + (m + 1) * P]

            def wr_(k, m):
                return wrb[:, k * H + m * P: k * H + (m + 1) * P]

            def wef(k, mz):
                return weffb[:, k * Z + mz * P: k * Z + (mz + 1) * P]

            def rt(k):
                return rtb[:, k * BS:(k + 1) * BS]

            def cbias(m, t):
                return ctab[:, m * T + t: m * T + t + 1]

            # ---- PE chain: pin the whole matmul order explicitly ----
            pe_prev = [None]

            def mm(out, lhsT, rhs, start, stop):
                i = nc.tensor.matmul(out, lhsT, rhs, start=start, stop=stop)
                if pe_prev[0] is not None:
                    add_dep_helper(i.ins, pe_prev[0].ins, sync=False,
                                   reason="pin PE order")
                pe_prev[0] = i
                return i

            # HAM warmup while the preamble DMAs land (no DMA dependency)
            wps = pp.tile([P, P], F32, tag="wu", name="wups")
            for i in range(N_WARMUP):
                mm(wps[:], warmt[:], warmt[:], True, True)

            # rw[m] = (r @ Wr)[m-tile]; k2/k3 first (their wrb half and
            # rtb are the first DMAs to land)
            rwps = [pp.tile([P, BS], F32, tag=f"pa{m}", name=f"prw{m}")
                    for m in range(MH)]
            for k in (2, 3, 0, 1):
                for m in range(MH):
                    mm(rwps[m][:], wr_(k, m), rt(k),
                       start=(k == 2), stop=(k == 1))
            rw = [wp.tile([P, BS], BF, tag=f"rw{m}", name=f"rw{m}")
                  for m in range(MH)]
            for m in range(MH):
                if m % 2 == 0:
                    nc.scalar.activation(rw[m][:], rwps[m][:], AF.Copy)
                else:
                    nc.vector.tensor_copy(rw[m][:], rwps[m][:])

            # ---- the scan; state accumulates in ps_d across all steps ----
            dve_prev = [None]

            def dve(fn, *args, **kw):
                i = fn(*args, **kw)
                if dve_prev[0] is not None:
                    add_dep_helper(i.ins, dve_prev[0].ins, sync=False,
                                   reason="pin DVE order")
                dve_prev[0] = i
                return i

            ps_d = [pp.tile([P, BS], F32, tag=f"pd{mz}", name=f"pd{mz}")
                    for mz in range(MZ)]
            z = [z0b[:, k * BS:(k + 1) * BS] for k in range(KZ)]
            for t in range(T):
                s = t % 2
                c = t // 2
                if s == 0:
                    eps_prefetch(c + 4)
                et = eps_tiles[c]

                if t == 0:
                    # state init: ps_d = z0 (start=True sets has_written)
                    for mz in range(MZ):
                        mm(ps_d[mz][:], identb[:],
                           z0b[:, mz * BS:(mz + 1) * BS], True, False)

                # stage A: ps_a[m] = sum_k Wz[k,m].T @ z[k] + I@rw[m].
                # Order is tuned so (a) k1 matmuls don't outrun the z'[1]
                # cast of the previous step, (b) ps_a[m] tiles complete
                # early and staggered for the ACT/DVE evac pipeline.
                ps_a = [pp.tile([P, BS], F32, tag=f"pa{m}",
                                name=f"pa{m}_{t}") for m in range(MH)]

                def a_k(k, m, stop=False):
                    mm(ps_a[m][:], wz(k, m), z[k], k == 0, stop)

                def a_rw(m):
                    mm(ps_a[m][:], identb[:], rw[m][:], False, False)

                a_k(0, 0)
                a_rw(0)
                a_k(0, 1)
                a_rw(1)
                a_k(1, 0, True)
                a_k(1, 1, True)
                a_k(0, 2)
                a_rw(2)
                a_k(1, 2, True)
                a_k(0, 3)
                a_rw(3)
                a_k(1, 3, True)

                # evac: a[m] = relu(ps_a[m] + c[m,t]); ACT m even, DVE odd
                a = []
                for m in range(MH):
                    at = ap_.tile([P, BS], BF, tag=f"a{m}", name=f"a{m}_{t}")
                    if m % 2 == 0:
                        nc.scalar.activation(at[:], ps_a[m][:], AF.Relu,
                                             bias=cbias(m, t))
                    else:
                        dve(nc.vector.tensor_scalar, at[:], ps_a[m][:],
                            cbias(m, t), 0.0, op0=OP.add, op1=OP.max)
                    a.append(at)

                # state update: ps_d[mz] += I@eps'[t] + sum_k a[k]@W_eff
                for mz in range(MZ):
                    mm(ps_d[mz][:], identb[:],
                       et[:, s * ZB + mz * BS: s * ZB + (mz + 1) * BS],
                       False, False)
                zbt = sp.tile([P, ZB], BF, tag="zb", name=f"zb{t}", bufs=3)
                for mz in range(MZ):
                    for k in range(KH):
                        mm(ps_d[mz][:], wef(k, mz), a[k][:],
                           start=False, stop=(k == KH - 1))
                    dve(nc.vector.tensor_copy,
                        zbt[:, mz * BS:(mz + 1) * BS], ps_d[mz][:])

                z = [zbt[:, k * BS:(k + 1) * BS] for k in range(KZ)]
                if t == T - 1:
                    # split the last DMA per z-tile so the final transfer
                    # starts as soon as the first cast lands
                    nc.sync.dma_start(d_zs[t, :, :BS], zbt[:, :BS])
                    nc.sync.dma_start(d_zs[t, :, BS:], zbt[:, BS:])
                else:
                    nc.sync.dma_start(d_zs[t], zbt[:])

    nc.compile()
    return nc


def _get_nc():
    if "nc" not in _CACHE:
        _CACHE["nc"] = _build()
    return _CACHE["nc"]


def _ktile_merge(x, ktiles):
    """[ktiles*128, W] -> [128, ktiles*W] with k-tiles side by side."""
    w = x.shape[-1]
    return np.ascontiguousarray(
        x.reshape(ktiles, P, w).transpose(1, 0, 2).reshape(P, ktiles * w))


def _in_maps(inputs):
    f32 = lambda x: np.ascontiguousarray(np.asarray(x, dtype=np.float32))
    r = f32(inputs["r"])
    noise0 = f32(inputs["noise0"])
    noise = f32(inputs["noise"])
    W0 = f32(inputs["W0"]).astype(np.float64)
    b0 = f32(inputs["b0"]).astype(np.float64)
    Wh = f32(inputs["Wh"]).astype(np.float64)
    bh = f32(inputs["bh"]).astype(np.float64)
    Wo = f32(inputs["Wo"]).astype(np.float64)
    bo = f32(inputs["bo"]).astype(np.float64)
    Wt = f32(inputs["Wt"]).astype(np.float64)
    bt = f32(inputs["bt"]).astype(np.float64)

    # host-side weight folding (see module docstring)
    weff_dt = DT * (Wh @ Wh @ Wo)                        # [H, Z]
    const = (bh @ Wh + bh) @ Wo + bo                     # [Z]
    ts = np.arange(1, T + 1, dtype=np.float64) * DT
    temb = np.maximum(ts[:, None] * Wt[0][None, :] + bt[None, :], 0.0)
    c = temb @ W0 + b0                                   # [T, H]
    ctabb = np.ascontiguousarray(
        c.T.reshape(MH, P, T).transpose(1, 0, 2).reshape(P, MH * T)
    ).astype(np.float32)

    shared = {
        "identb": np.eye(P, dtype=np.float32).astype(BF16),
        "wrb": _ktile_merge(W0[Z:], KR).astype(BF16),
        "weffb": _ktile_merge(weff_dt, KH).astype(BF16),
        "ctabb": ctabb,
    }
    wzm = _ktile_merge(W0[:Z], KZ).astype(BF16)
    rT = np.ascontiguousarray(r.T)                       # [R, B]
    z0T = np.ascontiguousarray(noise0.T)                 # [Z, B]
    # eps'[t] = sqrt_dt*eps + dt*const, feature-major, 2-step chunks
    epsp = (SQDT * noise.transpose(0, 2, 1)
            + DT * const.astype(np.float32)[None, :, None])  # [T, Z, B]
    maps = []
    for cix in range(NC):
        sl = slice(cix * BS, (cix + 1) * BS)
        m = dict(shared)
        m["rtb"] = _ktile_merge(
            np.ascontiguousarray(rT[:, sl]), KR).astype(BF16)
        z0m = _ktile_merge(
            np.ascontiguousarray(z0T[:, sl]), KZ).astype(BF16)
        m["pkb"] = np.ascontiguousarray(np.concatenate([wzm, z0m], axis=1))
        ec = np.ascontiguousarray(epsp[:, :, sl])        # [T, Z, BS]
        m["epsb"] = np.ascontiguousarray(
            ec.reshape(NCH, 2, KZ, P, BS).transpose(0, 3, 1, 2, 4)
            .reshape(NCH, P, 2 * ZB)).astype(BF16)
        maps.append(m)
    return maps, noise0, noise


def _run(inputs, **run_kwargs):
    from concourse.bass_utils import run_bass_kernel_spmd
    nc = _get_nc()
    maps, noise0, noise = _in_maps(inputs)
    res = run_bass_kernel_spmd(nc, maps, core_ids=list(range(NC)),
                               **run_kwargs)
    out = np.empty((3, T + 1, B, Z), np.float32)
    out[0, 0] = noise0
    out[1, 0] = 0.0
    out[2, 0] = 1.0
    out[2, 1:] = np.float32(SQDT)
    for cix in range(NC):
        sl = slice(cix * BS, (cix + 1) * BS)
        zs = res.results[cix]["zsb"].astype(np.float32)
        # [T, P, (mz, b)] -> [T, BS, Z]
        zs = (zs.reshape(T, P, KZ, BS).transpose(0, 3, 2, 1)
              .reshape(T, BS, Z))
        out[0, 1:, sl, :] = zs
    # mu = z - sqrt_dt*eps, exactly (reference: z = mu + sqrt_dt*eps)
    out[1, 1:] = out[0, 1:] - np.float32(SQDT) * noise
    return out, res


def kernel(**inputs) -> np.ndarray:
    out, _ = _run(inputs)
    return out


# revision 18
# speedup vs baseline: 1.0642x; 1.0642x over previous
"""Trainium2 Bass kernel for the DiffusionProcess problem (v4).

Strategy (hardcoded for B=2048, R=512, Z=256, H=512, T=16, 8 cores):
  - Data parallel: batch sharded 8 x 256, weights replicated.
  - Feature-major layout: activations [feature, batch]; matmuls
    out[M,N] = W[K,M].T @ x[K,N] with K,M tiles of 128, N = 256.
  - All matmul operands bf16 (1 row/cycle on the PE vs ~1.3 for f32r);
    PSUM accumulation stays f32.
  - Host-side weight folding (pure weight preprocessing):
      W_eff = dt * (Wh @ Wh @ Wo)            (no relu between them)
      const = (bh@Wh + bh)@Wo + bo
      c[t]  = temb_t @ W0 + b0               (per-step ACT bias columns)
      eps'[t] = sqrt_dt*eps[t] + dt*const    (folded into noise stream)
  - THE SCAN STATE LIVES IN PSUM: ps_d[mz] is initialized once with
    I@z0 (start=True) and then only accumulated into (start=False) for
    all 16 steps: each step adds I@eps'[t] plus sum_k a[k]@W_eff[k,mz].
    The f32 PSUM accumulation means the state is never rounded; the
    only per-step evacuation is a bf16 cast (DVE) that feeds the next
    step's matmuls and the zs output DMA.
  - Per-step device work:
      ps_a[m] = I@rw[m] + z_bf16@Wz[:,m]     (rw = r@Wr, on-device once)
      a[m]    = relu(ps_a[m] + c[m,t])       (ACT m=0,2 / DVE m=1,3)
      ps_d[mz]+= I@eps'[t,mz] + sum_k a[k] @ W_eff[k,mz]
      z_bf16  = cast(ps_d)                   (DVE, matmul rhs + output)
  - mus is NOT computed on device: mu = z - sqrt_dt*eps exactly, so the
    host reconstructs it from zs and the noise input. sigmas are
    constant. Only zs streams out (bf16), one DMA per step on the sync
    queue (whose HWDGE ring is otherwise idle during the scan).
  - Step-boundary retiming: stage-S is z-tile-major so cast(z'[0]) lands
    while S still works on z'[1]; the next step opens with 4 rw-inject
    matmuls (step-invariant operands) to cover the DVE tail latency.
  - HAM warmup: dummy matmuls keep the PE busy during the preamble DMA
    phase so the scan starts at the full 2.4 GHz clock; preamble DMAs
    are ordered so the rw dependencies (wrb, rtb) land first.
"""

import sys

if "/opt/trn_rl_repo" not in sys.path:
    sys.path.insert(0, "/opt/trn_rl_repo")

import numpy as np
import ml_dtypes

B, R, Z, H = 2048, 512, 256, 512
ZR = Z + R
T = 16
NC = 8
BS = B // NC          # 256 batch per core
DT = 1.0 / T
SQDT = DT ** 0.5
P = 128
KZ = Z // P           # 2
KR = R // P           # 4
KH = H // P           # 4
MH = H // P           # 4
MZ = Z // P           # 2
ZB = KZ * BS          # 512 = one step of z, feature-major columns
NCH = T // 2          # 8 two-step chunks for the eps input stream
BF16 = ml_dtypes.bfloat16

N_WARMUP = 26         # PE warmup matmuls during preamble DMA

_CACHE = {}


def _build():
    import concourse.bacc as bacc
    import concourse.tile as tile
    from concourse import mybir
    from concourse.tile_rust import add_dep_helper

    F32 = mybir.dt.float32
    BF = mybir.dt.bfloat16
    AF = mybir.ActivationFunctionType
    OP = mybir.AluOpType

    nc = bacc.Bacc("TRN2", target_bir_lowering=False, debug=False,
                   num_devices=NC)

    d_id = nc.dram_tensor("identb", [P, P], BF, kind="ExternalInput").ap()
    # pkb packs wzb [128, KZ*H] and z0b [128, ZB] into one DMA
    d_pk = nc.dram_tensor("pkb", [P, KZ * H + ZB], BF,
                          kind="ExternalInput").ap()
    d_wr = nc.dram_tensor("wrb", [P, KR * H], BF, kind="ExternalInput").ap()
    d_we = nc.dram_tensor("weffb", [P, KH * Z], BF,
                          kind="ExternalInput").ap()
    d_rt = nc.dram_tensor("rtb", [P, KR * BS], BF,
                          kind="ExternalInput").ap()
    d_ct = nc.dram_tensor("ctabb", [P, MH * T], F32,
                          kind="ExternalInput").ap()
    d_eps = nc.dram_tensor("epsb", [NCH, P, 2 * ZB], BF,
                           kind="ExternalInput").ap()
    d_zs = nc.dram_tensor("zsb", [T, P, ZB], BF,
                          kind="ExternalOutput").ap()

    with tile.TileContext(nc) as tc:
        with tc.tile_pool(name="w", bufs=1) as wp, \
             tc.tile_pool(name="act", bufs=2) as ap_, \
             tc.tile_pool(name="st", bufs=2) as sp, \
             tc.tile_pool(name="ps", bufs=1, space="PSUM") as pp:

            # ---- preamble DMA. Each ~256KB load occupies its HWDGE queue
            # for ~3.5us, so the rw deps (wrb halves, rtb) go FIRST on
            # separate queues and rw consumes the halves in arrival order.
            identb = wp.tile([P, P], BF, tag="identb", name="identb")
            nc.sync.dma_start(identb[:], d_id[:])
            wrb = wp.tile([P, KR * H], BF, tag="wrb", name="wrb")
            nc.sync.dma_start(wrb[:, :2 * H], d_wr[:, :2 * H])

            rtb = wp.tile([P, KR * BS], BF, tag="rtb", name="rtb")
            nc.scalar.dma_start(rtb[:], d_rt[:])
            nc.scalar.dma_start(wrb[:, 2 * H:], d_wr[:, 2 * H:])
            ctab = wp.tile([P, MH * T], F32, tag="ctab", name="ctab")
            nc.scalar.dma_start(ctab[:], d_ct[:])

            pkb = wp.tile([P, KZ * H + ZB], BF, tag="pkb", name="pkb")
            nc.gpsimd.dma_start(pkb[:], d_pk[:])
            wzb = pkb[:, :KZ * H]
            z0b = pkb[:, KZ * H:]

            weffb = wp.tile([P, KH * Z], BF, tag="weffb", name="weffb")
            nc.sync.dma_start(weffb[:], d_we[:])

            # one-time ucode/table loads, off the critical path
            dumv = wp.tile([P, 8], F32, tag="dum", name="dum")
            nc.scalar.activation(dumv[:, 0:1], identb[:, 0:1], AF.Relu)
            nc.vector.tensor_copy(dumv[:, 1:2], identb[:, 1:2])

            # eps prefetch ring on the gpsimd SWDGE queue
            eps_tiles = {}

            def eps_prefetch(c):
                if c < NCH:
                    et = sp.tile([P, 2 * ZB], BF, tag="eps",
                                 name=f"eps_{c}", bufs=5)
                    nc.gpsimd.dma_start(et[:], d_eps[c])
                    eps_tiles[c] = et

            for c in range(4):
                eps_prefetch(c)

            def wz(k, m):
                return wzb[:, k * H + m * P: k * H + (m + 1) * P]

            def wr_(k, m):
                return wrb[:, k * H + m * P: k * H + (m + 1) * P]

            def wef(k, mz):
                return weffb[:, k * Z + mz * P: k * Z + (mz + 1) * P]

            def rt(k):
                return rtb[:, k * BS:(k + 1) * BS]

            def cbias(m, t):
                return ctab[:, m * T + t: m * T + t + 1]

            # ---- PE chain: pin the whole matmul order explicitly ----
            pe_prev = [None]

            def mm(out, lhsT, rhs, start, stop):
                i = nc.tensor.matmul(out, lhsT, rhs, start=start, stop=stop)
                if pe_prev[0] is not None:
                    add_dep_helper(i.ins, pe_prev[0].ins, sync=False,
                                   reason="pin PE order")
                pe_prev[0] = i
                return i

            # HAM warmup while the preamble DMAs land (no DMA dependency)
            wps = pp.tile([P, P], F32, tag="wu", name="wups")
            for i in range(N_WARMUP):
                mm(wps[:], identb[:], identb[:], True, True)

            # rw[m] = (r @ Wr)[m-tile]; k0/k1 first (their wrb half lands
            # first, on the sync queue)
            rwps = [pp.tile([P, BS], F32, tag=f"pa{m}", name=f"prw{m}")
                    for m in range(MH)]
            for k in range(KR):
                for m in range(MH):
                    mm(rwps[m][:], wr_(k, m), rt(k),
                       start=(k == 0), stop=(k == KR - 1))
            rw = [wp.tile([P, BS], BF, tag=f"rw{m}", name=f"rw{m}")
                  for m in range(MH)]
            for m in range(MH):
                if m % 2 == 0:
                    nc.scalar.activation(rw[m][:], rwps[m][:], AF.Copy)
                else:
                    nc.vector.tensor_copy(rw[m][:], rwps[m][:])

            # ---- the scan; state accumulates in ps_d across all steps ----
            dve_prev = [None]

            def dve(fn, *args, **kw):
                i = fn(*args, **kw)
                if dve_prev[0] is not None:
                    add_dep_helper(i.ins, dve_prev[0].ins, sync=False,
                                   reason="pin DVE order")
                dve_prev[0] = i
                return i

            ps_d = [pp.tile([P, BS], F32, tag=f"pd{mz}", name=f"pd{mz}")
                    for mz in range(MZ)]
            z = [z0b[:, k * BS:(k + 1) * BS] for k in range(KZ)]
            for t in range(T):
                s = t % 2
                c = t // 2
                if s == 0:
                    eps_prefetch(c + 4)
                et = eps_tiles[c]

                if t == 0:
                    # state init: ps_d = z0 (start=True sets has_written)
                    for mz in range(MZ):
                        mm(ps_d[mz][:], identb[:],
                           z0b[:, mz * BS:(mz + 1) * BS], True, False)

                # stage A: ps_a[m] = sum_k Wz[k,m].T @ z[k] + I@rw[m].
                # Order is tuned so (a) k1 matmuls don't outrun the z'[1]
                # cast of the previous step, (b) ps_a[m] tiles complete
                # early and staggered for the ACT/DVE evac pipeline.
                ps_a = [pp.tile([P, BS], F32, tag=f"pa{m}",
                                name=f"pa{m}_{t}") for m in range(MH)]

                def a_k(k, m, stop=False):
                    mm(ps_a[m][:], wz(k, m), z[k], k == 0, stop)

                def a_rw(m):
                    mm(ps_a[m][:], identb[:], rw[m][:], False, False)

                a_k(0, 0)
                a_rw(0)
                a_k(0, 1)
                a_rw(1)
                a_k(1, 0, True)
                a_k(1, 1, True)
                a_k(0, 2)
                a_rw(2)
                a_k(1, 2, True)
                a_k(0, 3)
                a_rw(3)
                a_k(1, 3, True)

                # evac: a[m] = relu(ps_a[m] + c[m,t]); ACT m even, DVE odd
                a = []
                for m in range(MH):
                    at = ap_.tile([P, BS], BF, tag=f"a{m}", name=f"a{m}_{t}")
                    if m % 2 == 0:
                        nc.scalar.activation(at[:], ps_a[m][:], AF.Relu,
                                             bias=cbias(m, t))
                    else:
                        dve(nc.vector.tensor_scalar, at[:], ps_a[m][:],
                            cbias(m, t), 0.0, op0=OP.add, op1=OP.max)
                    a.append(at)

                # state update: ps_d[mz] += I@eps'[t] + sum_k a[k]@W_eff
                for mz in range(MZ):
                    mm(ps_d[mz][:], identb[:],
                       et[:, s * ZB + mz * BS: s * ZB + (mz + 1) * BS],
                       False, False)
                zbt = sp.tile([P, ZB], BF, tag="zb", name=f"zb{t}", bufs=3)
                for mz in range(MZ):
                    for k in range(KH):
                        mm(ps_d[mz][:], wef(k, mz), a[k][:],
                           start=False, stop=(k == KH - 1))
                    dve(nc.vector.tensor_copy,
                        zbt[:, mz * BS:(mz + 1) * BS], ps_d[mz][:])

                z = [zbt[:, k * BS:(k + 1) * BS] for k in range(KZ)]
                if t == T - 1:
                    # split the last DMA per z-tile so the final transfer
                    # starts as soon as the first cast lands
                    nc.sync.dma_start(d_zs[t, :, :BS], zbt[:, :BS])
                    nc.sync.dma_start(d_zs[t, :, BS:], zbt[:, BS:])
                else:
                    nc.sync.dma_start(d_zs[t], zbt[:])

    nc.compile()
    return nc


def _get_nc():
    if "nc" not in _CACHE:
        _CACHE["nc"] = _build()
    return _CACHE["nc"]


def _ktile_merge(x, ktiles):
    """[ktiles*128, W] -> [128, ktiles*W] with k-tiles side by side."""
    w = x.shape[-1]
    return np.ascontiguousarray(
        x.reshape(ktiles, P, w).transpose(1, 0, 2).reshape(P, ktiles * w))


def _in_maps(inputs):
    f32 = lambda x: np.ascontiguousarray(np.asarray(x, dtype=np.float32))
    r = f32(inputs["r"])
    noise0 = f32(inputs["noise0"])
    noise = f32(inputs["noise"])
    W0 = f32(inputs["W0"]).astype(np.float64)
    b0 = f32(inputs["b0"]).astype(np.float64)
    Wh = f32(inputs["Wh"]).astype(np.float64)
    bh = f32(inputs["bh"]).astype(np.float64)
    Wo = f32(inputs["Wo"]).astype(np.float64)
    bo = f32(inputs["bo"]).astype(np.float64)
    Wt = f32(inputs["Wt"]).astype(np.float64)
    bt = f32(inputs["bt"]).astype(np.float64)

    # host-side weight folding (see module docstring)
    weff_dt = DT * (Wh @ Wh @ Wo)                        # [H, Z]
    const = (bh @ Wh + bh) @ Wo + bo                     # [Z]
    ts = np.arange(1, T + 1, dtype=np.float64) * DT
    temb = np.maximum(ts[:, None] * Wt[0][None, :] + bt[None, :], 0.0)
    c = temb @ W0 + b0                                   # [T, H]
    ctabb = np.ascontiguousarray(
        c.T.reshape(MH, P, T).transpose(1, 0, 2).reshape(P, MH * T)
    ).astype(np.float32)

    shared = {
        "identb": np.eye(P, dtype=np.float32).astype(BF16),
        "wrb": _ktile_merge(W0[Z:], KR).astype(BF16),
        "weffb": _ktile_merge(weff_dt, KH).astype(BF16),
        "ctabb": ctabb,
    }
    wzm = _ktile_merge(W0[:Z], KZ).astype(BF16)
    rT = np.ascontiguousarray(r.T)                       # [R, B]
    z0T = np.ascontiguousarray(noise0.T)                 # [Z, B]
    # eps'[t] = sqrt_dt*eps + dt*const, feature-major, 2-step chunks
    epsp = (SQDT * noise.transpose(0, 2, 1)
            + DT * const.astype(np.float32)[None, :, None])  # [T, Z, B]
    maps = []
    for cix in range(NC):
        sl = slice(cix * BS, (cix + 1) * BS)
        m = dict(shared)
        m["rtb"] = _ktile_merge(
            np.ascontiguousarray(rT[:, sl]), KR).astype(BF16)
        z0m = _ktile_merge(
            np.ascontiguousarray(z0T[:, sl]), KZ).astype(BF16)
        m["pkb"] = np.ascontiguousarray(np.concatenate([wzm, z0m], axis=1))
        ec = np.ascontiguousarray(epsp[:, :, sl])        # [T, Z, BS]
        m["epsb"] = np.ascontiguousarray(
            ec.reshape(NCH, 2, KZ, P, BS).transpose(0, 3, 1, 2, 4)
            .reshape(NCH, P, 2 * ZB)).astype(BF16)
        maps.append(m)
    return maps, noise0, noise


def _run(inputs, **run_kwargs):
    from concourse.bass_utils import run_bass_kernel_spmd
    nc = _get_nc()
    maps, noise0, noise = _in_maps(inputs)
    res = run_bass_kernel_spmd(nc, maps, core_ids=list(range(NC)),
                               **run_kwargs)
    out = np.empty((3, T + 1, B, Z), np.float32)
    out[0, 0] = noise0
    out[1, 0] = 0.0
    out[2, 0] = 1.0
    out[2, 1:] = np.float32(SQDT)
    for cix in range(NC):
        sl = slice(cix * BS, (cix + 1) * BS)
        zs = res.results[cix]["zsb"].astype(np.float32)
        # [T, P, (mz, b)] -> [T, BS, Z]
        zs = (zs.reshape(T, P, KZ, BS).transpose(0, 3, 2, 1)
              .reshape(T, BS, Z))
        out[0, 1:, sl, :] = zs
    # mu = z - sqrt_dt*eps, exactly (reference: z = mu + sqrt_dt*eps)
    out[1, 1:] = out[0, 1:] - np.float32(SQDT) * noise
    return out, res


def kernel(**inputs) -> np.ndarray:
    out, _ = _run(inputs)
    return out


# revision 19
# speedup vs baseline: 1.0736x; 1.0088x over previous
"""Trainium2 Bass kernel for the DiffusionProcess problem (v4).

Strategy (hardcoded for B=2048, R=512, Z=256, H=512, T=16, 8 cores):
  - Data parallel: batch sharded 8 x 256, weights replicated.
  - Feature-major layout: activations [feature, batch]; matmuls
    out[M,N] = W[K,M].T @ x[K,N] with K,M tiles of 128, N = 256.
  - All matmul operands bf16 (1 row/cycle on the PE vs ~1.3 for f32r);
    PSUM accumulation stays f32.
  - Host-side weight folding (pure weight preprocessing):
      W_eff = dt * (Wh @ Wh @ Wo)            (no relu between them)
      const = (bh@Wh + bh)@Wo + bo
      c[t]  = temb_t @ W0 + b0               (per-step ACT bias columns)
      eps'[t] = sqrt_dt*eps[t] + dt*const    (folded into noise stream)
  - THE SCAN STATE LIVES IN PSUM: ps_d[mz] is initialized once with
    I@z0 (start=True) and then only accumulated into (start=False) for
    all 16 steps: each step adds I@eps'[t] plus sum_k a[k]@W_eff[k,mz].
    The f32 PSUM accumulation means the state is never rounded; the
    only per-step evacuation is a bf16 cast (DVE) that feeds the next
    step's matmuls and the zs output DMA.
  - Per-step device work:
      ps_a[m] = I@rw[m] + z_bf16@Wz[:,m]     (rw = r@Wr, on-device once)
      a[m]    = relu(ps_a[m] + c[m,t])       (ACT m=0,2 / DVE m=1,3)
      ps_d[mz]+= I@eps'[t,mz] + sum_k a[k] @ W_eff[k,mz]
      z_bf16  = cast(ps_d)                   (DVE, matmul rhs + output)
  - mus is NOT computed on device: mu = z - sqrt_dt*eps exactly, so the
    host reconstructs it from zs and the noise input. sigmas are
    constant. Only zs streams out (bf16), one DMA per step on the sync
    queue (whose HWDGE ring is otherwise idle during the scan).
  - Step-boundary retiming: stage-S is z-tile-major so cast(z'[0]) lands
    while S still works on z'[1]; the next step opens with 4 rw-inject
    matmuls (step-invariant operands) to cover the DVE tail latency.
  - HAM warmup: dummy matmuls keep the PE busy during the preamble DMA
    phase so the scan starts at the full 2.4 GHz clock; preamble DMAs
    are ordered so the rw dependencies (wrb, rtb) land first.
"""

import sys

if "/opt/trn_rl_repo" not in sys.path:
    sys.path.insert(0, "/opt/trn_rl_repo")

import numpy as np
import ml_dtypes

B, R, Z, H = 2048, 512, 256, 512
ZR = Z + R
T = 16
NC = 8
BS = B // NC          # 256 batch per core
DT = 1.0 / T
SQDT = DT ** 0.5
P = 128
KZ = Z // P           # 2
KR = R // P           # 4
KH = H // P           # 4
MH = H // P           # 4
MZ = Z // P           # 2
ZB = KZ * BS          # 512 = one step of z, feature-major columns
NCH = T // 2          # 8 two-step chunks for the eps input stream
BF16 = ml_dtypes.bfloat16

N_WARMUP = 26         # PE warmup matmuls during preamble DMA

_CACHE = {}


def _build():
    import concourse.bacc as bacc
    import concourse.tile as tile
    from concourse import mybir
    from concourse.tile_rust import add_dep_helper

    F32 = mybir.dt.float32
    BF = mybir.dt.bfloat16
    AF = mybir.ActivationFunctionType
    OP = mybir.AluOpType

    nc = bacc.Bacc("TRN2", target_bir_lowering=False, debug=False,
                   num_devices=NC)

    d_id = nc.dram_tensor("identb", [P, P], BF, kind="ExternalInput").ap()
    d_wz = nc.dram_tensor("wzb", [P, KZ * H], BF, kind="ExternalInput").ap()
    d_z0 = nc.dram_tensor("z0bb", [P, ZB], BF, kind="ExternalInput").ap()
    d_wr = nc.dram_tensor("wrb", [P, KR * H], BF, kind="ExternalInput").ap()
    d_we = nc.dram_tensor("weffb", [P, KH * Z], BF,
                          kind="ExternalInput").ap()
    d_rt = nc.dram_tensor("rtb", [P, KR * BS], BF,
                          kind="ExternalInput").ap()
    d_ct = nc.dram_tensor("ctabb", [P, MH * T], F32,
                          kind="ExternalInput").ap()
    d_eps = nc.dram_tensor("epsb", [NCH, P, 2 * ZB], BF,
                           kind="ExternalInput").ap()
    d_zs = nc.dram_tensor("zsb", [T, P, ZB], BF,
                          kind="ExternalOutput").ap()

    with tile.TileContext(nc) as tc:
        with tc.tile_pool(name="w", bufs=1) as wp, \
             tc.tile_pool(name="act", bufs=2) as ap_, \
             tc.tile_pool(name="st", bufs=2) as sp, \
             tc.tile_pool(name="ps", bufs=1, space="PSUM") as pp:

            # ---- preamble DMA. Each ~256KB load occupies its HWDGE queue
            # for ~3.5us, so the rw deps (wrb halves, rtb) go FIRST on
            # separate queues and rw consumes the halves in arrival order.
            identb = wp.tile([P, P], BF, tag="identb", name="identb")
            nc.sync.dma_start(identb[:], d_id[:])
            wrb = wp.tile([P, KR * H], BF, tag="wrb", name="wrb")
            nc.sync.dma_start(wrb[:, :2 * H], d_wr[:, :2 * H])

            rtb = wp.tile([P, KR * BS], BF, tag="rtb", name="rtb")
            nc.scalar.dma_start(rtb[:], d_rt[:])
            nc.scalar.dma_start(wrb[:, 2 * H:], d_wr[:, 2 * H:])
            ctab = wp.tile([P, MH * T], F32, tag="ctab", name="ctab")
            nc.scalar.dma_start(ctab[:], d_ct[:])

            wzb = wp.tile([P, KZ * H], BF, tag="wzb", name="wzb")
            nc.gpsimd.dma_start(wzb[:], d_wz[:])
            z0b = wp.tile([P, ZB], BF, tag="z0b", name="z0b")
            nc.gpsimd.dma_start(z0b[:], d_z0[:])

            weffb = wp.tile([P, KH * Z], BF, tag="weffb", name="weffb")
            nc.sync.dma_start(weffb[:], d_we[:])

            # one-time ucode/table loads, off the critical path
            dumv = wp.tile([P, 8], F32, tag="dum", name="dum")
            nc.scalar.activation(dumv[:, 0:1], identb[:, 0:1], AF.Relu)
            nc.vector.tensor_copy(dumv[:, 1:2], identb[:, 1:2])

            # eps prefetch ring on the gpsimd SWDGE queue
            eps_tiles = {}

            def eps_prefetch(c):
                if c < NCH:
                    et = sp.tile([P, 2 * ZB], BF, tag="eps",
                                 name=f"eps_{c}", bufs=5)
                    nc.gpsimd.dma_start(et[:], d_eps[c])
                    eps_tiles[c] = et

            for c in range(4):
                eps_prefetch(c)

            def wz(k, m):
                return wzb[:, k * H + m * P: k * H + (m + 1) * P]

            def wr_(k, m):
                return wrb[:, k * H + m * P: k * H + (m + 1) * P]

            def wef(k, mz):
                return weffb[:, k * Z + mz * P: k * Z + (mz + 1) * P]

            def rt(k):
                return rtb[:, k * BS:(k + 1) * BS]

            def cbias(m, t):
                return ctab[:, m * T + t: m * T + t + 1]

            # ---- PE chain: pin the whole matmul order explicitly ----
            pe_prev = [None]

            def mm(out, lhsT, rhs, start, stop):
                i = nc.tensor.matmul(out, lhsT, rhs, start=start, stop=stop)
                if pe_prev[0] is not None:
                    add_dep_helper(i.ins, pe_prev[0].ins, sync=False,
                                   reason="pin PE order")
                pe_prev[0] = i
                return i

            # HAM warmup while the preamble DMAs land (no DMA dependency)
            wps = pp.tile([P, P], F32, tag="wu", name="wups")
            for i in range(N_WARMUP):
                mm(wps[:], identb[:], identb[:], True, True)

            # rw[m] = (r @ Wr)[m-tile]; k0/k1 first (their wrb half lands
            # first, on the sync queue)
            rwps = [pp.tile([P, BS], F32, tag=f"pa{m}", name=f"prw{m}")
                    for m in range(MH)]
            for k in range(KR):
                for m in range(MH):
                    mm(rwps[m][:], wr_(k, m), rt(k),
                       start=(k == 0), stop=(k == KR - 1))
            rw = [wp.tile([P, BS], BF, tag=f"rw{m}", name=f"rw{m}")
                  for m in range(MH)]
            for m in range(MH):
                if m % 2 == 0:
                    nc.scalar.activation(rw[m][:], rwps[m][:], AF.Copy)
                else:
                    nc.vector.tensor_copy(rw[m][:], rwps[m][:])

            # ---- the scan; state accumulates in ps_d across all steps ----
            dve_prev = [None]

            def dve(fn, *args, **kw):
                i = fn(*args, **kw)
                if dve_prev[0] is not None:
                    add_dep_helper(i.ins, dve_prev[0].ins, sync=False,
                                   reason="pin DVE order")
                dve_prev[0] = i
                return i

            ps_d = [pp.tile([P, BS], F32, tag=f"pd{mz}", name=f"pd{mz}")
                    for mz in range(MZ)]
            z = [z0b[:, k * BS:(k + 1) * BS] for k in range(KZ)]
            for t in range(T):
                s = t % 2
                c = t // 2
                if s == 0:
                    eps_prefetch(c + 4)
                et = eps_tiles[c]

                if t == 0:
                    # state init: ps_d = z0 (start=True sets has_written)
                    for mz in range(MZ):
                        mm(ps_d[mz][:], identb[:],
                           z0b[:, mz * BS:(mz + 1) * BS], True, False)

                # stage A: ps_a[m] = sum_k Wz[k,m].T @ z[k] + I@rw[m].
                # Order is tuned so (a) k1 matmuls don't outrun the z'[1]
                # cast of the previous step, (b) ps_a[m] tiles complete
                # early and staggered for the ACT/DVE evac pipeline.
                ps_a = [pp.tile([P, BS], F32, tag=f"pa{m}",
                                name=f"pa{m}_{t}") for m in range(MH)]

                def a_k(k, m, stop=False):
                    mm(ps_a[m][:], wz(k, m), z[k], k == 0, stop)

                def a_rw(m):
                    mm(ps_a[m][:], identb[:], rw[m][:], False, False)

                a_k(0, 0)
                a_rw(0)
                a_k(0, 1)
                a_rw(1)
                a_k(1, 0, True)
                a_k(1, 1, True)
                a_k(0, 2)
                a_rw(2)
                a_k(1, 2, True)
                a_k(0, 3)
                a_rw(3)
                a_k(1, 3, True)

                # evac: a[m] = relu(ps_a[m] + c[m,t]); ACT m even, DVE odd
                a = []
                for m in range(MH):
                    at = ap_.tile([P, BS], BF, tag=f"a{m}", name=f"a{m}_{t}")
                    if m % 2 == 0:
                        nc.scalar.activation(at[:], ps_a[m][:], AF.Relu,
                                             bias=cbias(m, t))
                    else:
                        dve(nc.vector.tensor_scalar, at[:], ps_a[m][:],
                            cbias(m, t), 0.0, op0=OP.add, op1=OP.max)
                    a.append(at)

                # state update: ps_d[mz] += I@eps'[t] + sum_k a[k]@W_eff
                for mz in range(MZ):
                    mm(ps_d[mz][:], identb[:],
                       et[:, s * ZB + mz * BS: s * ZB + (mz + 1) * BS],
                       False, False)
                zbt = sp.tile([P, ZB], BF, tag="zb", name=f"zb{t}", bufs=3)
                for mz in range(MZ):
                    for k in range(KH):
                        mm(ps_d[mz][:], wef(k, mz), a[k][:],
                           start=False, stop=(k == KH - 1))
                    dve(nc.vector.tensor_copy,
                        zbt[:, mz * BS:(mz + 1) * BS], ps_d[mz][:])

                z = [zbt[:, k * BS:(k + 1) * BS] for k in range(KZ)]
                if t == T - 1:
                    # split the last DMA per z-tile so the final transfer
                    # starts as soon as the first cast lands
                    nc.sync.dma_start(d_zs[t, :, :BS], zbt[:, :BS])
                    nc.sync.dma_start(d_zs[t, :, BS:], zbt[:, BS:])
                else:
                    nc.sync.dma_start(d_zs[t], zbt[:])

    nc.compile()
    return nc


def _get_nc():
    if "nc" not in _CACHE:
        _CACHE["nc"] = _build()
    return _CACHE["nc"]


def _ktile_merge(x, ktiles):
    """[ktiles*128, W] -> [128, ktiles*W] with k-tiles side by side."""
    w = x.shape[-1]
    return np.ascontiguousarray(
        x.reshape(ktiles, P, w).transpose(1, 0, 2).reshape(P, ktiles * w))


def _in_maps(inputs):
    f32 = lambda x: np.ascontiguousarray(np.asarray(x, dtype=np.float32))
    r = f32(inputs["r"])
    noise0 = f32(inputs["noise0"])
    noise = f32(inputs["noise"])
    W0 = f32(inputs["W0"]).astype(np.float64)
    b0 = f32(inputs["b0"]).astype(np.float64)
    Wh = f32(inputs["Wh"]).astype(np.float64)
    bh = f32(inputs["bh"]).astype(np.float64)
    Wo = f32(inputs["Wo"]).astype(np.float64)
    bo = f32(inputs["bo"]).astype(np.float64)
    Wt = f32(inputs["Wt"]).astype(np.float64)
    bt = f32(inputs["bt"]).astype(np.float64)

    # host-side weight folding (see module docstring)
    weff_dt = DT * (Wh @ Wh @ Wo)                        # [H, Z]
    const = (bh @ Wh + bh) @ Wo + bo                     # [Z]
    ts = np.arange(1, T + 1, dtype=np.float64) * DT
    temb = np.maximum(ts[:, None] * Wt[0][None, :] + bt[None, :], 0.0)
    c = temb @ W0 + b0                                   # [T, H]
    ctabb = np.ascontiguousarray(
        c.T.reshape(MH, P, T).transpose(1, 0, 2).reshape(P, MH * T)
    ).astype(np.float32)

    shared = {
        "identb": np.eye(P, dtype=np.float32).astype(BF16),
        "wrb": _ktile_merge(W0[Z:], KR).astype(BF16),
        "wzb": _ktile_merge(W0[:Z], KZ).astype(BF16),
        "weffb": _ktile_merge(weff_dt, KH).astype(BF16),
        "ctabb": ctabb,
    }

    rT = np.ascontiguousarray(r.T)                       # [R, B]
    z0T = np.ascontiguousarray(noise0.T)                 # [Z, B]
    # eps'[t] = sqrt_dt*eps + dt*const, feature-major, 2-step chunks
    epsp = (SQDT * noise.transpose(0, 2, 1)
            + DT * const.astype(np.float32)[None, :, None])  # [T, Z, B]
    maps = []
    for cix in range(NC):
        sl = slice(cix * BS, (cix + 1) * BS)
        m = dict(shared)
        m["rtb"] = _ktile_merge(
            np.ascontiguousarray(rT[:, sl]), KR).astype(BF16)
        m["z0bb"] = _ktile_merge(
            np.ascontiguousarray(z0T[:, sl]), KZ).astype(BF16)
        ec = np.ascontiguousarray(epsp[:, :, sl])        # [T, Z, BS]
        m["epsb"] = np.ascontiguousarray(
            ec.reshape(NCH, 2, KZ, P, BS).transpose(0, 3, 1, 2, 4)
            .reshape(NCH, P, 2 * ZB)).astype(BF16)
        maps.append(m)
    return maps, noise0, noise


def _run(inputs, **run_kwargs):
    from concourse.bass_utils import run_bass_kernel_spmd
    nc = _get_nc()
    maps, noise0, noise = _in_maps(inputs)
    res = run_bass_kernel_spmd(nc, maps, core_ids=list(range(NC)),
                               **run_kwargs)
    out = np.empty((3, T + 1, B, Z), np.float32)
    out[0, 0] = noise0
    out[1, 0] = 0.0
    out[2, 0] = 1.0
    out[2, 1:] = np.float32(SQDT)
    for cix in range(NC):
        sl = slice(cix * BS, (cix + 1) * BS)
        zs = res.results[cix]["zsb"].astype(np.float32)
        # [T, P, (mz, b)] -> [T, BS, Z]
        zs = (zs.reshape(T, P, KZ, BS).transpose(0, 3, 2, 1)
              .reshape(T, BS, Z))
        out[0, 1:, sl, :] = zs
    # mu = z - sqrt_dt*eps, exactly (reference: z = mu + sqrt_dt*eps)
    out[1, 1:] = out[0, 1:] - np.float32(SQDT) * noise
    return out, res


def kernel(**inputs) -> np.ndarray:
    out, _ = _run(inputs)
    return out


# revision 20
# speedup vs baseline: 1.0790x; 1.0051x over previous
"""Trainium2 Bass kernel for the DiffusionProcess problem (v4).

Strategy (hardcoded for B=2048, R=512, Z=256, H=512, T=16, 8 cores):
  - Data parallel: batch sharded 8 x 256, weights replicated.
  - Feature-major layout: activations [feature, batch]; matmuls
    out[M,N] = W[K,M].T @ x[K,N] with K,M tiles of 128, N = 256.
  - All matmul operands bf16 (1 row/cycle on the PE vs ~1.3 for f32r);
    PSUM accumulation stays f32.
  - Host-side weight folding (pure weight preprocessing):
      W_eff = dt * (Wh @ Wh @ Wo)            (no relu between them)
      const = (bh@Wh + bh)@Wo + bo
      c[t]  = temb_t @ W0 + b0               (per-step ACT bias columns)
      eps'[t] = sqrt_dt*eps[t] + dt*const    (folded into noise stream)
  - THE SCAN STATE LIVES IN PSUM: ps_d[mz] is initialized once with
    I@z0 (start=True) and then only accumulated into (start=False) for
    all 16 steps: each step adds I@eps'[t] plus sum_k a[k]@W_eff[k,mz].
    The f32 PSUM accumulation means the state is never rounded; the
    only per-step evacuation is a bf16 cast (DVE) that feeds the next
    step's matmuls and the zs output DMA.
  - Per-step device work:
      ps_a[m] = I@rw[m] + z_bf16@Wz[:,m]     (rw = r@Wr, on-device once)
      a[m]    = relu(ps_a[m] + c[m,t])       (ACT m=0,2 / DVE m=1,3)
      ps_d[mz]+= I@eps'[t,mz] + sum_k a[k] @ W_eff[k,mz]
      z_bf16  = cast(ps_d)                   (DVE, matmul rhs + output)
  - mus is NOT computed on device: mu = z - sqrt_dt*eps exactly, so the
    host reconstructs it from zs and the noise input. sigmas are
    constant. Only zs streams out (bf16), one DMA per step on the sync
    queue (whose HWDGE ring is otherwise idle during the scan).
  - Step-boundary retiming: stage-S is z-tile-major so cast(z'[0]) lands
    while S still works on z'[1]; the next step opens with 4 rw-inject
    matmuls (step-invariant operands) to cover the DVE tail latency.
  - HAM warmup: dummy matmuls keep the PE busy during the preamble DMA
    phase so the scan starts at the full 2.4 GHz clock; preamble DMAs
    are ordered so the rw dependencies (wrb, rtb) land first.
"""

import sys

if "/opt/trn_rl_repo" not in sys.path:
    sys.path.insert(0, "/opt/trn_rl_repo")

import numpy as np
import ml_dtypes

B, R, Z, H = 2048, 512, 256, 512
ZR = Z + R
T = 16
NC = 8
BS = B // NC          # 256 batch per core
DT = 1.0 / T
SQDT = DT ** 0.5
P = 128
KZ = Z // P           # 2
KR = R // P           # 4
KH = H // P           # 4
MH = H // P           # 4
MZ = Z // P           # 2
ZB = KZ * BS          # 512 = one step of z, feature-major columns
NCH = T // 2          # 8 two-step chunks for the eps input stream
BF16 = ml_dtypes.bfloat16

N_WARMUP = 26         # PE warmup matmuls during preamble DMA

_CACHE = {}


def _build():
    import concourse.bacc as bacc
    import concourse.tile as tile
    from concourse import mybir
    from concourse.tile_rust import add_dep_helper

    F32 = mybir.dt.float32
    BF = mybir.dt.bfloat16
    AF = mybir.ActivationFunctionType
    OP = mybir.AluOpType

    nc = bacc.Bacc("TRN2", target_bir_lowering=False, debug=False,
                   num_devices=NC)

    d_id = nc.dram_tensor("identb", [P, P], BF, kind="ExternalInput").ap()
    d_wz = nc.dram_tensor("wzb", [P, KZ * H], BF, kind="ExternalInput").ap()
    d_z0 = nc.dram_tensor("z0bb", [P, ZB], BF, kind="ExternalInput").ap()
    d_wr = nc.dram_tensor("wrb", [P, KR * H], BF, kind="ExternalInput").ap()
    d_we = nc.dram_tensor("weffb", [P, KH * Z], BF,
                          kind="ExternalInput").ap()
    d_rt = nc.dram_tensor("rtb", [P, KR * BS], BF,
                          kind="ExternalInput").ap()
    d_ct = nc.dram_tensor("ctabb", [P, MH * T], F32,
                          kind="ExternalInput").ap()
    d_eps = nc.dram_tensor("epsb", [NCH, P, 2 * ZB], BF,
                           kind="ExternalInput").ap()
    d_zs = nc.dram_tensor("zsb", [T, P, ZB], BF,
                          kind="ExternalOutput").ap()

    with tile.TileContext(nc) as tc:
        with tc.tile_pool(name="w", bufs=1) as wp, \
             tc.tile_pool(name="act", bufs=2) as ap_, \
             tc.tile_pool(name="st", bufs=2) as sp, \
             tc.tile_pool(name="ps", bufs=1, space="PSUM") as pp:

            # ---- preamble DMA. Each ~256KB load occupies its HWDGE queue
            # for ~3.5us, so the rw deps (wrb halves, rtb) go FIRST on
            # separate queues and rw consumes the halves in arrival order.
            identb = wp.tile([P, P], BF, tag="identb", name="identb")
            nc.sync.dma_start(identb[:], d_id[:])
            wrb = wp.tile([P, KR * H], BF, tag="wrb", name="wrb")
            nc.sync.dma_start(wrb[:, :2 * H], d_wr[:, :2 * H])

            rtb = wp.tile([P, KR * BS], BF, tag="rtb", name="rtb")
            nc.scalar.dma_start(rtb[:], d_rt[:])
            nc.scalar.dma_start(wrb[:, 2 * H:], d_wr[:, 2 * H:])
            ctab = wp.tile([P, MH * T], F32, tag="ctab", name="ctab")
            nc.scalar.dma_start(ctab[:], d_ct[:])

            wzb = wp.tile([P, KZ * H], BF, tag="wzb", name="wzb")
            nc.gpsimd.dma_start(wzb[:], d_wz[:])
            z0b = wp.tile([P, ZB], BF, tag="z0b", name="z0b")
            nc.gpsimd.dma_start(z0b[:], d_z0[:])

            weffb = wp.tile([P, KH * Z], BF, tag="weffb", name="weffb")
            nc.sync.dma_start(weffb[:], d_we[:])

            # one-time ucode/table loads, off the critical path
            dumv = wp.tile([P, 8], F32, tag="dum", name="dum")
            nc.scalar.activation(dumv[:, 0:1], identb[:, 0:1], AF.Relu)
            nc.vector.tensor_copy(dumv[:, 1:2], identb[:, 1:2])

            # eps prefetch ring on the gpsimd SWDGE queue
            eps_tiles = {}

            def eps_prefetch(c):
                if c < NCH:
                    et = sp.tile([P, 2 * ZB], BF, tag="eps",
                                 name=f"eps_{c}", bufs=5)
                    nc.gpsimd.dma_start(et[:], d_eps[c])
                    eps_tiles[c] = et

            for c in range(4):
                eps_prefetch(c)

            def wz(k, m):
                return wzb[:, k * H + m * P: k * H + (m + 1) * P]

            def wr_(k, m):
                return wrb[:, k * H + m * P: k * H + (m + 1) * P]

            def wef(k, mz):
                return weffb[:, k * Z + mz * P: k * Z + (mz + 1) * P]

            def rt(k):
                return rtb[:, k * BS:(k + 1) * BS]

            def cbias(m, t):
                return ctab[:, m * T + t: m * T + t + 1]

            # ---- PE chain: pin the whole matmul order explicitly ----
            pe_prev = [None]

            def mm(out, lhsT, rhs, start, stop):
                i = nc.tensor.matmul(out, lhsT, rhs, start=start, stop=stop)
                if pe_prev[0] is not None:
                    add_dep_helper(i.ins, pe_prev[0].ins, sync=False,
                                   reason="pin PE order")
                pe_prev[0] = i
                return i

            # HAM warmup while the preamble DMAs land (no DMA dependency)
            wps = pp.tile([P, P], F32, tag="wu", name="wups")
            for i in range(N_WARMUP):
                mm(wps[:], identb[:], identb[:], True, True)

            # rw[m] = (r @ Wr)[m-tile]; k0/k1 first (their wrb half lands
            # first, on the sync queue)
            rwps = [pp.tile([P, BS], F32, tag=f"pa{m}", name=f"prw{m}")
                    for m in range(MH)]
            for k in range(KR):
                for m in range(MH):
                    mm(rwps[m][:], wr_(k, m), rt(k),
                       start=(k == 0), stop=(k == KR - 1))
            rw = [wp.tile([P, BS], BF, tag=f"rw{m}", name=f"rw{m}")
                  for m in range(MH)]
            for m in range(MH):
                if m % 2 == 0:
                    nc.scalar.activation(rw[m][:], rwps[m][:], AF.Copy)
                else:
                    nc.vector.tensor_copy(rw[m][:], rwps[m][:])

            # ---- the scan; state accumulates in ps_d across all steps ----
            dve_prev = [None]

            def dve(fn, *args, **kw):
                i = fn(*args, **kw)
                if dve_prev[0] is not None:
                    add_dep_helper(i.ins, dve_prev[0].ins, sync=False,
                                   reason="pin DVE order")
                dve_prev[0] = i
                return i

            # Persistent PSUM state: ps_d[mz] holds z(t) in f32 across all
            # steps; ps_a[m] holds (z-basis@Wz + rw) and is advanced each
            # step DIFFERENTIALLY with dz@Wz, where dz(t)=ps_d(t)-zb(t-1)
            # is the step increment (DVE, bf16). This removes the 4 rw
            # injects per step and takes the output cast (now on ACT) off
            # the recurrence path entirely.
            ps_d = [pp.tile([P, BS], F32, tag=f"pd{mz}", name=f"pd{mz}")
                    for mz in range(MZ)]
            ps_a = [pp.tile([P, BS], F32, tag=f"pa{m}", name=f"pa{m}")
                    for m in range(MH)]
            dz_prev = None
            zb_prev = z0b
            for t in range(T):
                s = t % 2
                c = t // 2
                if s == 0:
                    eps_prefetch(c + 4)
                et = eps_tiles[c]

                if t == 0:
                    # state init: ps_d = z0 (start=True sets has_written)
                    for mz in range(MZ):
                        mm(ps_d[mz][:], identb[:],
                           z0b[:, mz * BS:(mz + 1) * BS], True, False)
                    # ps_a init: z0@Wz + rw (order: per-m completion
                    # staggered for the evac pipeline)
                    mm(ps_a[0][:], wz(0, 0), z0b[:, :BS], True, False)
                    mm(ps_a[0][:], identb[:], rw[0][:], False, False)
                    mm(ps_a[1][:], wz(0, 1), z0b[:, :BS], True, False)
                    mm(ps_a[1][:], identb[:], rw[1][:], False, False)
                    mm(ps_a[0][:], wz(1, 0), z0b[:, BS:], False, True)
                    mm(ps_a[1][:], wz(1, 1), z0b[:, BS:], False, True)
                    mm(ps_a[2][:], wz(0, 2), z0b[:, :BS], True, False)
                    mm(ps_a[2][:], identb[:], rw[2][:], False, False)
                    mm(ps_a[2][:], wz(1, 2), z0b[:, BS:], False, True)
                    mm(ps_a[3][:], wz(0, 3), z0b[:, :BS], True, False)
                    mm(ps_a[3][:], identb[:], rw[3][:], False, False)
                    mm(ps_a[3][:], wz(1, 3), z0b[:, BS:], False, True)
                else:
                    # stage A: ps_a[m] += sum_k Wz[k,m].T @ dz[k]
                    for m in range(MH):
                        mm(ps_a[m][:], wz(0, m), dz_prev[:, :BS],
                           False, False)
                    for m in range(MH):
                        mm(ps_a[m][:], wz(1, m), dz_prev[:, BS:],
                           False, True)

                # evac: a[m] = relu(ps_a[m] + c[m,t]); ACT m even, DVE odd
                a = []
                for m in range(MH):
                    at = ap_.tile([P, BS], BF, tag=f"a{m}", name=f"a{m}_{t}")
                    if m % 2 == 0:
                        nc.scalar.activation(at[:], ps_a[m][:], AF.Relu,
                                             bias=cbias(m, t))
                    else:
                        dve(nc.vector.tensor_scalar, at[:], ps_a[m][:],
                            cbias(m, t), 0.0, op0=OP.add, op1=OP.max)
                    a.append(at)

                # state update: ps_d[mz] += I@eps'[t] + sum_k a[k]@W_eff
                for mz in range(MZ):
                    mm(ps_d[mz][:], identb[:],
                       et[:, s * ZB + mz * BS: s * ZB + (mz + 1) * BS],
                       False, False)
                zbt = sp.tile([P, ZB], BF, tag="zb", name=f"zb{t}", bufs=3)
                dzt = sp.tile([P, ZB], BF, tag="dz", name=f"dz{t}", bufs=2)
                for mz in range(MZ):
                    sl = slice(mz * BS, (mz + 1) * BS)
                    for k in range(KH):
                        mm(ps_d[mz][:], wef(k, mz), a[k][:],
                           start=False, stop=(k == KH - 1))
                    # recurrence path: dz on DVE, straight from PSUM
                    dve(nc.vector.tensor_sub, dzt[:, sl], ps_d[mz][:],
                        zb_prev[:, sl])
                    # output path: bf16 cast on ACT, off the recurrence
                    nc.scalar.activation(zbt[:, sl], ps_d[mz][:], AF.Copy)

                dz_prev = dzt[:]
                zb_prev = zbt[:]
                if t == T - 1:
                    # split the last DMA per z-tile so the final transfer
                    # starts as soon as the first cast lands
                    nc.sync.dma_start(d_zs[t, :, :BS], zbt[:, :BS])
                    nc.sync.dma_start(d_zs[t, :, BS:], zbt[:, BS:])
                else:
                    nc.sync.dma_start(d_zs[t], zbt[:])

    nc.compile()
    return nc


def _get_nc():
    if "nc" not in _CACHE:
        _CACHE["nc"] = _build()
    return _CACHE["nc"]


def _ktile_merge(x, ktiles):
    """[ktiles*128, W] -> [128, ktiles*W] with k-tiles side by side."""
    w = x.shape[-1]
    return np.ascontiguousarray(
        x.reshape(ktiles, P, w).transpose(1, 0, 2).reshape(P, ktiles * w))


def _in_maps(inputs):
    f32 = lambda x: np.ascontiguousarray(np.asarray(x, dtype=np.float32))
    r = f32(inputs["r"])
    noise0 = f32(inputs["noise0"])
    noise = f32(inputs["noise"])
    W0 = f32(inputs["W0"]).astype(np.float64)
    b0 = f32(inputs["b0"]).astype(np.float64)
    Wh = f32(inputs["Wh"]).astype(np.float64)
    bh = f32(inputs["bh"]).astype(np.float64)
    Wo = f32(inputs["Wo"]).astype(np.float64)
    bo = f32(inputs["bo"]).astype(np.float64)
    Wt = f32(inputs["Wt"]).astype(np.float64)
    bt = f32(inputs["bt"]).astype(np.float64)

    # host-side weight folding (see module docstring)
    weff_dt = DT * (Wh @ Wh @ Wo)                        # [H, Z]
    const = (bh @ Wh + bh) @ Wo + bo                     # [Z]
    ts = np.arange(1, T + 1, dtype=np.float64) * DT
    temb = np.maximum(ts[:, None] * Wt[0][None, :] + bt[None, :], 0.0)
    c = temb @ W0 + b0                                   # [T, H]
    ctabb = np.ascontiguousarray(
        c.T.reshape(MH, P, T).transpose(1, 0, 2).reshape(P, MH * T)
    ).astype(np.float32)

    shared = {
        "identb": np.eye(P, dtype=np.float32).astype(BF16),
        "wrb": _ktile_merge(W0[Z:], KR).astype(BF16),
        "wzb": _ktile_merge(W0[:Z], KZ).astype(BF16),
        "weffb": _ktile_merge(weff_dt, KH).astype(BF16),
        "ctabb": ctabb,
    }

    rT = np.ascontiguousarray(r.T)                       # [R, B]
    z0T = np.ascontiguousarray(noise0.T)                 # [Z, B]
    # eps'[t] = sqrt_dt*eps + dt*const, feature-major, 2-step chunks
    epsp = (SQDT * noise.transpose(0, 2, 1)
            + DT * const.astype(np.float32)[None, :, None])  # [T, Z, B]
    maps = []
    for cix in range(NC):
        sl = slice(cix * BS, (cix + 1) * BS)
        m = dict(shared)
        m["rtb"] = _ktile_merge(
            np.ascontiguousarray(rT[:, sl]), KR).astype(BF16)
        m["z0bb"] = _ktile_merge(
            np.ascontiguousarray(z0T[:, sl]), KZ).astype(BF16)
        ec = np.ascontiguousarray(epsp[:, :, sl])        # [T, Z, BS]
        m["epsb"] = np.ascontiguousarray(
            ec.reshape(NCH, 2, KZ, P, BS).transpose(0, 3, 1, 2, 4)
            .reshape(NCH, P, 2 * ZB)).astype(BF16)
        maps.append(m)
    return maps, noise0, noise


def _run(inputs, **run_kwargs):
    from concourse.bass_utils import run_bass_kernel_spmd
    nc = _get_nc()
    maps, noise0, noise = _in_maps(inputs)
    res = run_bass_kernel_spmd(nc, maps, core_ids=list(range(NC)),
                               **run_kwargs)
    out = np.empty((3, T + 1, B, Z), np.float32)
    out[0, 0] = noise0
    out[1, 0] = 0.0
    out[2, 0] = 1.0
    out[2, 1:] = np.float32(SQDT)
    for cix in range(NC):
        sl = slice(cix * BS, (cix + 1) * BS)
        zs = res.results[cix]["zsb"].astype(np.float32)
        # [T, P, (mz, b)] -> [T, BS, Z]
        zs = (zs.reshape(T, P, KZ, BS).transpose(0, 3, 2, 1)
              .reshape(T, BS, Z))
        out[0, 1:, sl, :] = zs
    # mu = z - sqrt_dt*eps, exactly (reference: z = mu + sqrt_dt*eps)
    out[1, 1:] = out[0, 1:] - np.float32(SQDT) * noise
    return out, res


def kernel(**inputs) -> np.ndarray:
    out, _ = _run(inputs)
    return out


# revision 22
# speedup vs baseline: 1.1065x; 1.0255x over previous
"""Trainium2 Bass kernel for the DiffusionProcess problem (v4).

Strategy (hardcoded for B=2048, R=512, Z=256, H=512, T=16, 8 cores):
  - Data parallel: batch sharded 8 x 256, weights replicated.
  - Feature-major layout: activations [feature, batch]; matmuls
    out[M,N] = W[K,M].T @ x[K,N] with K,M tiles of 128, N = 256.
  - All matmul operands bf16 (1 row/cycle on the PE vs ~1.3 for f32r);
    PSUM accumulation stays f32.
  - Host-side weight folding (pure weight preprocessing):
      W_eff = dt * (Wh @ Wh @ Wo)            (no relu between them)
      const = (bh@Wh + bh)@Wo + bo
      c[t]  = temb_t @ W0 + b0               (per-step ACT bias columns)
      eps'[t] = sqrt_dt*eps[t] + dt*const    (folded into noise stream)
  - THE SCAN STATE LIVES IN PSUM: ps_d[mz] is initialized once with
    I@z0 (start=True) and then only accumulated into (start=False) for
    all 16 steps: each step adds I@eps'[t] plus sum_k a[k]@W_eff[k,mz].
    The f32 PSUM accumulation means the state is never rounded; the
    only per-step evacuation is a bf16 cast (DVE) that feeds the next
    step's matmuls and the zs output DMA.
  - Per-step device work:
      ps_a[m] = I@rw[m] + z_bf16@Wz[:,m]     (rw = r@Wr, on-device once)
      a[m]    = relu(ps_a[m] + c[m,t])       (ACT m=0,2 / DVE m=1,3)
      ps_d[mz]+= I@eps'[t,mz] + sum_k a[k] @ W_eff[k,mz]
      z_bf16  = cast(ps_d)                   (DVE, matmul rhs + output)
  - mus is NOT computed on device: mu = z - sqrt_dt*eps exactly, so the
    host reconstructs it from zs and the noise input. sigmas are
    constant. Only zs streams out (bf16), one DMA per step on the sync
    queue (whose HWDGE ring is otherwise idle during the scan).
  - Step-boundary retiming: stage-S is z-tile-major so cast(z'[0]) lands
    while S still works on z'[1]; the next step opens with 4 rw-inject
    matmuls (step-invariant operands) to cover the DVE tail latency.
  - HAM warmup: dummy matmuls keep the PE busy during the preamble DMA
    phase so the scan starts at the full 2.4 GHz clock; preamble DMAs
    are ordered so the rw dependencies (wrb, rtb) land first.
"""

import sys

if "/opt/trn_rl_repo" not in sys.path:
    sys.path.insert(0, "/opt/trn_rl_repo")

import numpy as np
import ml_dtypes

B, R, Z, H = 2048, 512, 256, 512
ZR = Z + R
T = 16
NC = 8
BS = B // NC          # 256 batch per core
DT = 1.0 / T
SQDT = DT ** 0.5
P = 128
KZ = Z // P           # 2
KR = R // P           # 4
KH = H // P           # 4
MH = H // P           # 4
MZ = Z // P           # 2
ZB = KZ * BS          # 512 = one step of z, feature-major columns
NCH = T // 2          # 8 two-step chunks for the eps input stream
BF16 = ml_dtypes.bfloat16

N_WARMUP = 26         # PE warmup matmuls during preamble DMA

_CACHE = {}


def _build():
    import concourse.bacc as bacc
    import concourse.tile as tile
    from concourse import mybir
    from concourse.tile_rust import add_dep_helper

    F32 = mybir.dt.float32
    BF = mybir.dt.bfloat16
    AF = mybir.ActivationFunctionType
    OP = mybir.AluOpType

    nc = bacc.Bacc("TRN2", target_bir_lowering=False, debug=False,
                   num_devices=NC)

    d_id = nc.dram_tensor("identb", [P, P], BF, kind="ExternalInput").ap()
    d_wz = nc.dram_tensor("wzb", [P, KZ * H], BF, kind="ExternalInput").ap()
    d_z0 = nc.dram_tensor("z0bb", [P, ZB], BF, kind="ExternalInput").ap()
    d_wr = nc.dram_tensor("wrb", [P, KR * H], BF, kind="ExternalInput").ap()
    d_we = nc.dram_tensor("weffb", [P, KH * Z], BF,
                          kind="ExternalInput").ap()
    d_rt = nc.dram_tensor("rtb", [P, KR * BS], BF,
                          kind="ExternalInput").ap()
    d_ct = nc.dram_tensor("ctabb", [P, MH * T], F32,
                          kind="ExternalInput").ap()
    d_eps = nc.dram_tensor("epsb", [NCH, P, 2 * ZB], BF,
                           kind="ExternalInput").ap()
    d_zs = nc.dram_tensor("zsb", [T, P, ZB], BF,
                          kind="ExternalOutput").ap()

    with tile.TileContext(nc) as tc:
        with tc.tile_pool(name="w", bufs=1) as wp, \
             tc.tile_pool(name="act", bufs=2) as ap_, \
             tc.tile_pool(name="st", bufs=2) as sp, \
             tc.tile_pool(name="ps", bufs=1, space="PSUM") as pp:

            # ---- preamble DMA. Each ~256KB load occupies its HWDGE queue
            # for ~3.5us, so the rw deps (wrb halves, rtb) go FIRST on
            # separate queues and rw consumes the halves in arrival order.
            identb = wp.tile([P, P], BF, tag="identb", name="identb")
            nc.sync.dma_start(identb[:], d_id[:])
            wrb = wp.tile([P, KR * H], BF, tag="wrb", name="wrb")
            nc.sync.dma_start(wrb[:, :2 * H], d_wr[:, :2 * H])

            rtb = wp.tile([P, KR * BS], BF, tag="rtb", name="rtb")
            nc.scalar.dma_start(rtb[:], d_rt[:])
            nc.scalar.dma_start(wrb[:, 2 * H:], d_wr[:, 2 * H:])
            ctab = wp.tile([P, MH * T], F32, tag="ctab", name="ctab")
            nc.scalar.dma_start(ctab[:], d_ct[:])

            wzb = wp.tile([P, KZ * H], BF, tag="wzb", name="wzb")
            nc.gpsimd.dma_start(wzb[:], d_wz[:])
            z0b = wp.tile([P, ZB], BF, tag="z0b", name="z0b")
            nc.gpsimd.dma_start(z0b[:], d_z0[:])

            weffb = wp.tile([P, KH * Z], BF, tag="weffb", name="weffb")
            nc.sync.dma_start(weffb[:], d_we[:])

            # one-time ucode/table loads, off the critical path
            dumv = wp.tile([P, 8], F32, tag="dum", name="dum")
            nc.scalar.activation(dumv[:, 0:1], identb[:, 0:1], AF.Relu)
            nc.vector.tensor_copy(dumv[:, 1:2], identb[:, 1:2])

            # eps prefetch ring on the gpsimd SWDGE queue
            eps_tiles = {}

            def eps_prefetch(c):
                if c < NCH:
                    et = sp.tile([P, 2 * ZB], BF, tag="eps",
                                 name=f"eps_{c}", bufs=5)
                    nc.gpsimd.dma_start(et[:], d_eps[c])
                    eps_tiles[c] = et

            for c in range(4):
                eps_prefetch(c)

            def wz(k, m):
                return wzb[:, k * H + m * P: k * H + (m + 1) * P]

            def wr_(k, m):
                return wrb[:, k * H + m * P: k * H + (m + 1) * P]

            def wef(k, mz):
                return weffb[:, k * Z + mz * P: k * Z + (mz + 1) * P]

            def rt(k):
                return rtb[:, k * BS:(k + 1) * BS]

            def cbias(m, t):
                return ctab[:, m * T + t: m * T + t + 1]

            # ---- PE chain: pin the whole matmul order explicitly ----
            pe_prev = [None]

            def mm(out, lhsT, rhs, start, stop):
                i = nc.tensor.matmul(out, lhsT, rhs, start=start, stop=stop)
                if pe_prev[0] is not None:
                    add_dep_helper(i.ins, pe_prev[0].ins, sync=False,
                                   reason="pin PE order")
                pe_prev[0] = i
                return i

            # HAM warmup while the preamble DMAs land (no DMA dependency)
            wps = pp.tile([P, P], F32, tag="wu", name="wups")
            for i in range(N_WARMUP):
                mm(wps[:], identb[:], identb[:], True, True)

            # rw[m] = (r @ Wr)[m-tile]; k0/k1 first (their wrb half lands
            # first, on the sync queue)
            rwps = [pp.tile([P, BS], F32, tag=f"pa{m}", name=f"prw{m}")
                    for m in range(MH)]
            for k in range(KR):
                for m in range(MH):
                    mm(rwps[m][:], wr_(k, m), rt(k),
                       start=(k == 0), stop=(k == KR - 1))
            rw = [wp.tile([P, BS], BF, tag=f"rw{m}", name=f"rw{m}")
                  for m in range(MH)]
            for m in range(MH):
                if m % 2 == 0:
                    nc.scalar.activation(rw[m][:], rwps[m][:], AF.Copy)
                else:
                    nc.vector.tensor_copy(rw[m][:], rwps[m][:])

            # ---- the scan; state accumulates in ps_d across all steps ----
            dve_prev = [None]

            def dve(fn, *args, **kw):
                i = fn(*args, **kw)
                if dve_prev[0] is not None:
                    add_dep_helper(i.ins, dve_prev[0].ins, sync=False,
                                   reason="pin DVE order")
                dve_prev[0] = i
                return i

            # Persistent PSUM state: ps_d[mz] holds z(t) in f32 across all
            # steps; ps_a[m] holds (z-basis@Wz + rw) and is advanced each
            # step DIFFERENTIALLY with dz@Wz, where dz(t)=ps_d(t)-zb(t-1)
            # is the step increment (DVE, bf16). This removes the 4 rw
            # injects per step and takes the output cast (now on ACT) off
            # the recurrence path entirely.
            ps_d = [pp.tile([P, BS], F32, tag=f"pd{mz}", name=f"pd{mz}")
                    for mz in range(MZ)]
            ps_a = [pp.tile([P, BS], F32, tag=f"pa{m}", name=f"pa{m}")
                    for m in range(MH)]
            dz_prev = None
            zb_prev = z0b
            for t in range(T):
                s = t % 2
                c = t // 2
                if s == 0:
                    eps_prefetch(c + 4)
                et = eps_tiles[c]

                if t == 0:
                    # state init: ps_d = z0 (start=True sets has_written)
                    for mz in range(MZ):
                        mm(ps_d[mz][:], identb[:],
                           z0b[:, mz * BS:(mz + 1) * BS], True, False)
                    # ps_a init: z0@Wz first (z0b/wzb land early), the
                    # rw injects last (wrb is the latest-arriving DMA)
                    for m in range(MH):
                        mm(ps_a[m][:], wz(0, m), z0b[:, :BS], True, False)
                    for m in range(MH):
                        mm(ps_a[m][:], wz(1, m), z0b[:, BS:], False, False)
                    for m in range(MH):
                        mm(ps_a[m][:], identb[:], rw[m][:], False, True)
                else:
                    # stage A: ps_a[m] += sum_k Wz[k,m].T @ dz[k]
                    for m in range(MH):
                        mm(ps_a[m][:], wz(0, m), dz_prev[:, :BS],
                           False, False)
                    for m in range(MH):
                        mm(ps_a[m][:], wz(1, m), dz_prev[:, BS:],
                           False, True)

                # evac: a[m] = relu(ps_a[m] + c[m,t]); ACT m even, DVE odd
                a = []
                for m in range(MH):
                    at = ap_.tile([P, BS], BF, tag=f"a{m}", name=f"a{m}_{t}")
                    if m % 2 == 0:
                        nc.scalar.activation(at[:], ps_a[m][:], AF.Relu,
                                             bias=cbias(m, t))
                    else:
                        dve(nc.vector.tensor_scalar, at[:], ps_a[m][:],
                            cbias(m, t), 0.0, op0=OP.add, op1=OP.max)
                    a.append(at)

                # state update: ps_d[mz] += I@eps'[t] + sum_k a[k]@W_eff
                for mz in range(MZ):
                    mm(ps_d[mz][:], identb[:],
                       et[:, s * ZB + mz * BS: s * ZB + (mz + 1) * BS],
                       False, False)
                zbt = sp.tile([P, ZB], BF, tag="zb", name=f"zb{t}", bufs=3)
                dzt = sp.tile([P, ZB], BF, tag="dz", name=f"dz{t}", bufs=2)
                for mz in range(MZ):
                    sl = slice(mz * BS, (mz + 1) * BS)
                    for k in range(KH):
                        mm(ps_d[mz][:], wef(k, mz), a[k][:],
                           start=False, stop=(k == KH - 1))
                    if t < T - 1:
                        # recurrence path: dz on DVE, straight from PSUM;
                        # output cast on ACT, off the recurrence
                        dve(nc.vector.tensor_sub, dzt[:, sl], ps_d[mz][:],
                            zb_prev[:, sl])
                        nc.scalar.activation(zbt[:, sl], ps_d[mz][:],
                                             AF.Copy)
                    else:
                        # last step: no dz needed - fast DVE cast so the
                        # final output DMA fires as early as possible
                        dve(nc.vector.tensor_copy, zbt[:, sl], ps_d[mz][:])

                dz_prev = dzt[:]
                zb_prev = zbt[:]
                if t == T - 1:
                    # split the last DMA per z-tile so the final transfer
                    # starts as soon as the first cast lands
                    nc.sync.dma_start(d_zs[t, :, :BS], zbt[:, :BS])
                    nc.sync.dma_start(d_zs[t, :, BS:], zbt[:, BS:])
                else:
                    nc.sync.dma_start(d_zs[t], zbt[:])

    nc.compile()
    return nc


def _get_nc():
    if "nc" not in _CACHE:
        _CACHE["nc"] = _build()
    return _CACHE["nc"]


def _ktile_merge(x, ktiles):
    """[ktiles*128, W] -> [128, ktiles*W] with k-tiles side by side."""
    w = x.shape[-1]
    return np.ascontiguousarray(
        x.reshape(ktiles, P, w).transpose(1, 0, 2).reshape(P, ktiles * w))


def _in_maps(inputs):
    f32 = lambda x: np.ascontiguousarray(np.asarray(x, dtype=np.float32))
    r = f32(inputs["r"])
    noise0 = f32(inputs["noise0"])
    noise = f32(inputs["noise"])
    W0 = f32(inputs["W0"]).astype(np.float64)
    b0 = f32(inputs["b0"]).astype(np.float64)
    Wh = f32(inputs["Wh"]).astype(np.float64)
    bh = f32(inputs["bh"]).astype(np.float64)
    Wo = f32(inputs["Wo"]).astype(np.float64)
    bo = f32(inputs["bo"]).astype(np.float64)
    Wt = f32(inputs["Wt"]).astype(np.float64)
    bt = f32(inputs["bt"]).astype(np.float64)

    # host-side weight folding (see module docstring)
    weff_dt = DT * (Wh @ Wh @ Wo)                        # [H, Z]
    const = (bh @ Wh + bh) @ Wo + bo                     # [Z]
    ts = np.arange(1, T + 1, dtype=np.float64) * DT
    temb = np.maximum(ts[:, None] * Wt[0][None, :] + bt[None, :], 0.0)
    c = temb @ W0 + b0                                   # [T, H]
    ctabb = np.ascontiguousarray(
        c.T.reshape(MH, P, T).transpose(1, 0, 2).reshape(P, MH * T)
    ).astype(np.float32)

    shared = {
        "identb": np.eye(P, dtype=np.float32).astype(BF16),
        "wrb": _ktile_merge(W0[Z:], KR).astype(BF16),
        "wzb": _ktile_merge(W0[:Z], KZ).astype(BF16),
        "weffb": _ktile_merge(weff_dt, KH).astype(BF16),
        "ctabb": ctabb,
    }

    rT = np.ascontiguousarray(r.T)                       # [R, B]
    z0T = np.ascontiguousarray(noise0.T)                 # [Z, B]
    # eps'[t] = sqrt_dt*eps + dt*const, feature-major, 2-step chunks
    epsp = (SQDT * noise.transpose(0, 2, 1)
            + DT * const.astype(np.float32)[None, :, None])  # [T, Z, B]
    maps = []
    for cix in range(NC):
        sl = slice(cix * BS, (cix + 1) * BS)
        m = dict(shared)
        m["rtb"] = _ktile_merge(
            np.ascontiguousarray(rT[:, sl]), KR).astype(BF16)
        m["z0bb"] = _ktile_merge(
            np.ascontiguousarray(z0T[:, sl]), KZ).astype(BF16)
        ec = np.ascontiguousarray(epsp[:, :, sl])        # [T, Z, BS]
        m["epsb"] = np.ascontiguousarray(
            ec.reshape(NCH, 2, KZ, P, BS).transpose(0, 3, 1, 2, 4)
            .reshape(NCH, P, 2 * ZB)).astype(BF16)
        maps.append(m)
    return maps, noise0, noise


def _run(inputs, **run_kwargs):
    from concourse.bass_utils import run_bass_kernel_spmd
    nc = _get_nc()
    maps, noise0, noise = _in_maps(inputs)
    res = run_bass_kernel_spmd(nc, maps, core_ids=list(range(NC)),
                               **run_kwargs)
    out = np.empty((3, T + 1, B, Z), np.float32)
    out[0, 0] = noise0
    out[1, 0] = 0.0
    out[2, 0] = 1.0
    out[2, 1:] = np.float32(SQDT)
    for cix in range(NC):
        sl = slice(cix * BS, (cix + 1) * BS)
        zs = res.results[cix]["zsb"].astype(np.float32)
        # [T, P, (mz, b)] -> [T, BS, Z]
        zs = (zs.reshape(T, P, KZ, BS).transpose(0, 3, 2, 1)
              .reshape(T, BS, Z))
        out[0, 1:, sl, :] = zs
    # mu = z - sqrt_dt*eps, exactly (reference: z = mu + sqrt_dt*eps)
    out[1, 1:] = out[0, 1:] - np.float32(SQDT) * noise
    return out, res


def kernel(**inputs) -> np.ndarray:
    out, _ = _run(inputs)
    return out


# revision 23
# speedup vs baseline: 1.1069x; 1.0003x over previous
"""Trainium2 Bass kernel for the DiffusionProcess problem (v4).

Strategy (hardcoded for B=2048, R=512, Z=256, H=512, T=16, 8 cores):
  - Data parallel: batch sharded 8 x 256, weights replicated.
  - Feature-major layout: activations [feature, batch]; matmuls
    out[M,N] = W[K,M].T @ x[K,N] with K,M tiles of 128, N = 256.
  - All matmul operands bf16 (1 row/cycle on the PE vs ~1.3 for f32r);
    PSUM accumulation stays f32.
  - Host-side weight folding (pure weight preprocessing):
      W_eff = dt * (Wh @ Wh @ Wo)            (no relu between them)
      const = (bh@Wh + bh)@Wo + bo
      c[t]  = temb_t @ W0 + b0               (per-step ACT bias columns)
      eps'[t] = sqrt_dt*eps[t] + dt*const    (folded into noise stream)
  - THE SCAN STATE LIVES IN PSUM: ps_d[mz] is initialized once with
    I@z0 (start=True) and then only accumulated into (start=False) for
    all 16 steps: each step adds I@eps'[t] plus sum_k a[k]@W_eff[k,mz].
    The f32 PSUM accumulation means the state is never rounded; the
    only per-step evacuation is a bf16 cast (DVE) that feeds the next
    step's matmuls and the zs output DMA.
  - Per-step device work:
      ps_a[m] = I@rw[m] + z_bf16@Wz[:,m]     (rw = r@Wr, on-device once)
      a[m]    = relu(ps_a[m] + c[m,t])       (ACT m=0,2 / DVE m=1,3)
      ps_d[mz]+= I@eps'[t,mz] + sum_k a[k] @ W_eff[k,mz]
      z_bf16  = cast(ps_d)                   (DVE, matmul rhs + output)
  - mus is NOT computed on device: mu = z - sqrt_dt*eps exactly, so the
    host reconstructs it from zs and the noise input. sigmas are
    constant. Only zs streams out (bf16), one DMA per step on the sync
    queue (whose HWDGE ring is otherwise idle during the scan).
  - Step-boundary retiming: stage-S is z-tile-major so cast(z'[0]) lands
    while S still works on z'[1]; the next step opens with 4 rw-inject
    matmuls (step-invariant operands) to cover the DVE tail latency.
  - HAM warmup: dummy matmuls keep the PE busy during the preamble DMA
    phase so the scan starts at the full 2.4 GHz clock; preamble DMAs
    are ordered so the rw dependencies (wrb, rtb) land first.
"""

import sys

if "/opt/trn_rl_repo" not in sys.path:
    sys.path.insert(0, "/opt/trn_rl_repo")

import numpy as np
import ml_dtypes

B, R, Z, H = 2048, 512, 256, 512
ZR = Z + R
T = 16
NC = 8
BS = B // NC          # 256 batch per core
DT = 1.0 / T
SQDT = DT ** 0.5
P = 128
KZ = Z // P           # 2
KR = R // P           # 4
KH = H // P           # 4
MH = H // P           # 4
MZ = Z // P           # 2
ZB = KZ * BS          # 512 = one step of z, feature-major columns
NCH = T // 2          # 8 two-step chunks for the eps input stream
BF16 = ml_dtypes.bfloat16

N_WARMUP = 26         # PE warmup matmuls during preamble DMA

_CACHE = {}


def _build():
    import concourse.bacc as bacc
    import concourse.tile as tile
    from concourse import mybir
    from concourse.tile_rust import add_dep_helper

    F32 = mybir.dt.float32
    BF = mybir.dt.bfloat16
    AF = mybir.ActivationFunctionType
    OP = mybir.AluOpType

    nc = bacc.Bacc("TRN2", target_bir_lowering=False, debug=False,
                   num_devices=NC)

    d_id = nc.dram_tensor("identb", [P, P], BF, kind="ExternalInput").ap()
    d_wz = nc.dram_tensor("wzb", [P, KZ * H], BF, kind="ExternalInput").ap()
    d_z0 = nc.dram_tensor("z0bb", [P, ZB], BF, kind="ExternalInput").ap()
    d_wr = nc.dram_tensor("wrb", [P, KR * H], BF, kind="ExternalInput").ap()
    d_we = nc.dram_tensor("weffb", [P, KH * Z], BF,
                          kind="ExternalInput").ap()
    d_rt = nc.dram_tensor("rtb", [P, KR * BS], BF,
                          kind="ExternalInput").ap()
    d_ct = nc.dram_tensor("ctabb", [P, MH * T], F32,
                          kind="ExternalInput").ap()
    d_eps = nc.dram_tensor("epsb", [NCH, P, 2 * ZB], BF,
                           kind="ExternalInput").ap()
    d_zs = nc.dram_tensor("zsb", [T, P, ZB], BF,
                          kind="ExternalOutput").ap()

    with tile.TileContext(nc) as tc:
        with tc.tile_pool(name="w", bufs=1) as wp, \
             tc.tile_pool(name="act", bufs=2) as ap_, \
             tc.tile_pool(name="st", bufs=2) as sp, \
             tc.tile_pool(name="ps", bufs=1, space="PSUM") as pp:

            # ---- preamble DMA. Each ~256KB load occupies its HWDGE queue
            # for ~3.5us, so the rw deps (wrb halves, rtb) go FIRST on
            # separate queues and rw consumes the halves in arrival order.
            identb = wp.tile([P, P], BF, tag="identb", name="identb")
            nc.sync.dma_start(identb[:], d_id[:])
            wrb = wp.tile([P, KR * H], BF, tag="wrb", name="wrb")
            nc.sync.dma_start(wrb[:, :2 * H], d_wr[:, :2 * H])

            rtb = wp.tile([P, KR * BS], BF, tag="rtb", name="rtb")
            nc.scalar.dma_start(rtb[:], d_rt[:])
            nc.scalar.dma_start(wrb[:, 2 * H:], d_wr[:, 2 * H:])
            ctab = wp.tile([P, MH * T], F32, tag="ctab", name="ctab")
            nc.scalar.dma_start(ctab[:], d_ct[:])

            wzb = wp.tile([P, KZ * H], BF, tag="wzb", name="wzb")
            nc.gpsimd.dma_start(wzb[:], d_wz[:])
            z0b = wp.tile([P, ZB], BF, tag="z0b", name="z0b")
            nc.gpsimd.dma_start(z0b[:], d_z0[:])

            weffb = wp.tile([P, KH * Z], BF, tag="weffb", name="weffb")
            nc.sync.dma_start(weffb[:], d_we[:])

            # one-time ucode/table loads, off the critical path
            dumv = wp.tile([P, 8], F32, tag="dum", name="dum")
            nc.scalar.activation(dumv[:, 0:1], identb[:, 0:1], AF.Relu)
            nc.vector.tensor_copy(dumv[:, 1:2], identb[:, 1:2])

            # eps prefetch ring on the gpsimd SWDGE queue
            eps_tiles = {}

            def eps_prefetch(c):
                if c < NCH:
                    et = sp.tile([P, 2 * ZB], BF, tag="eps",
                                 name=f"eps_{c}", bufs=5)
                    nc.gpsimd.dma_start(et[:], d_eps[c])
                    eps_tiles[c] = et

            for c in range(4):
                eps_prefetch(c)

            def wz(k, m):
                return wzb[:, k * H + m * P: k * H + (m + 1) * P]

            def wr_(k, m):
                return wrb[:, k * H + m * P: k * H + (m + 1) * P]

            def wef(k, mz):
                return weffb[:, k * Z + mz * P: k * Z + (mz + 1) * P]

            def rt(k):
                return rtb[:, k * BS:(k + 1) * BS]

            def cbias(m, t):
                return ctab[:, m * T + t: m * T + t + 1]

            # ---- PE chain: pin the whole matmul order explicitly ----
            pe_prev = [None]

            def mm(out, lhsT, rhs, start, stop):
                i = nc.tensor.matmul(out, lhsT, rhs, start=start, stop=stop)
                if pe_prev[0] is not None:
                    add_dep_helper(i.ins, pe_prev[0].ins, sync=False,
                                   reason="pin PE order")
                pe_prev[0] = i
                return i

            # HAM warmup while the preamble DMAs land (no DMA dependency)
            wps = pp.tile([P, P], F32, tag="wu", name="wups")
            for i in range(N_WARMUP):
                mm(wps[:], identb[:], identb[:], True, True)

            # rw[m] = (r @ Wr)[m-tile]; k0/k1 first (their wrb half lands
            # first, on the sync queue)
            rwps = [pp.tile([P, BS], F32, tag=f"pa{m}", name=f"prw{m}")
                    for m in range(MH)]
            for k in range(KR):
                for m in range(MH):
                    mm(rwps[m][:], wr_(k, m), rt(k),
                       start=(k == 0), stop=(k == KR - 1))
            rw = [wp.tile([P, BS], BF, tag=f"rw{m}", name=f"rw{m}")
                  for m in range(MH)]
            for m in range(MH):
                if m % 2 == 0:
                    nc.scalar.activation(rw[m][:], rwps[m][:], AF.Copy)
                else:
                    nc.vector.tensor_copy(rw[m][:], rwps[m][:])

            # ---- the scan; state accumulates in ps_d across all steps ----
            dve_prev = [None]

            def dve(fn, *args, **kw):
                i = fn(*args, **kw)
                if dve_prev[0] is not None:
                    add_dep_helper(i.ins, dve_prev[0].ins, sync=False,
                                   reason="pin DVE order")
                dve_prev[0] = i
                return i

            # Persistent PSUM state: ps_d[mz] holds z(t) in f32 across all
            # steps; ps_a[m] holds (z-basis@Wz + rw) and is advanced each
            # step DIFFERENTIALLY with dz@Wz, where dz(t)=ps_d(t)-zb(t-1)
            # is the step increment (DVE, bf16). This removes the 4 rw
            # injects per step and takes the output cast (now on ACT) off
            # the recurrence path entirely.
            ps_d = [pp.tile([P, BS], F32, tag=f"pd{mz}", name=f"pd{mz}")
                    for mz in range(MZ)]
            ps_a = [pp.tile([P, BS], F32, tag=f"pa{m}", name=f"pa{m}")
                    for m in range(MH)]
            dz_prev = None
            zb_prev = z0b
            for t in range(T):
                s = t % 2
                c = t // 2
                if s == 0:
                    eps_prefetch(c + 4)
                et = eps_tiles[c]

                if t == 0:
                    # state init: ps_d = z0 (start=True sets has_written)
                    for mz in range(MZ):
                        mm(ps_d[mz][:], identb[:],
                           z0b[:, mz * BS:(mz + 1) * BS], True, False)
                    # ps_a init: z0@Wz first (z0b/wzb land early), the
                    # rw injects last (wrb is the latest-arriving DMA)
                    for m in range(MH):
                        mm(ps_a[m][:], wz(0, m), z0b[:, :BS], True, False)
                    for m in range(MH):
                        mm(ps_a[m][:], wz(1, m), z0b[:, BS:], False, False)
                    for m in range(MH):
                        mm(ps_a[m][:], identb[:], rw[m][:], False, True)
                else:
                    # stage A: ps_a[m] += sum_k Wz[k,m].T @ dz[k]
                    for m in range(MH):
                        mm(ps_a[m][:], wz(0, m), dz_prev[:, :BS],
                           False, False)
                    for m in range(MH):
                        mm(ps_a[m][:], wz(1, m), dz_prev[:, BS:],
                           False, True)

                # evac: a[m] = relu(ps_a[m] + c[m,t]); ACT m even, DVE odd
                a = []
                for m in range(MH):
                    at = ap_.tile([P, BS], BF, tag=f"a{m}", name=f"a{m}_{t}")
                    if m % 2 == 0:
                        nc.scalar.activation(at[:], ps_a[m][:], AF.Relu,
                                             bias=cbias(m, t))
                    else:
                        dve(nc.vector.tensor_scalar, at[:], ps_a[m][:],
                            cbias(m, t), 0.0, op0=OP.add, op1=OP.max)
                    a.append(at)

                # state update: ps_d[mz] += I@eps'[t] + sum_k a[k]@W_eff
                for mz in range(MZ):
                    mm(ps_d[mz][:], identb[:],
                       et[:, s * ZB + mz * BS: s * ZB + (mz + 1) * BS],
                       False, False)
                zbt = sp.tile([P, ZB], BF, tag="zb", name=f"zb{t}", bufs=3)
                dzt = sp.tile([P, ZB], BF, tag="dz", name=f"dz{t}", bufs=2)
                for mz in range(MZ):
                    sl = slice(mz * BS, (mz + 1) * BS)
                    for k in range(KH):
                        mm(ps_d[mz][:], wef(k, mz), a[k][:],
                           start=False, stop=(k == KH - 1))
                    if t < T - 1:
                        # recurrence path: dz on DVE, straight from PSUM.
                        # PSUM is read by NOTHING else, so the next step's
                        # eps inject only waits on this early op.
                        dve(nc.vector.tensor_sub, dzt[:, sl], ps_d[mz][:],
                            zb_prev[:, sl])
                        # output path: zb = zb_prev + dz on the otherwise
                        # idle GPSIMD, fully off the recurrence
                        nc.gpsimd.tensor_add(zbt[:, sl], zb_prev[:, sl],
                                             dzt[:, sl])
                    else:
                        # last step: no dz needed - fast DVE cast so the
                        # final output DMA fires as early as possible
                        dve(nc.vector.tensor_copy, zbt[:, sl], ps_d[mz][:])

                dz_prev = dzt[:]
                zb_prev = zbt[:]
                if t == T - 1:
                    # split the last DMA per z-tile so the final transfer
                    # starts as soon as the first cast lands
                    nc.sync.dma_start(d_zs[t, :, :BS], zbt[:, :BS])
                    nc.sync.dma_start(d_zs[t, :, BS:], zbt[:, BS:])
                else:
                    nc.sync.dma_start(d_zs[t], zbt[:])

    nc.compile()
    return nc


def _get_nc():
    if "nc" not in _CACHE:
        _CACHE["nc"] = _build()
    return _CACHE["nc"]


def _ktile_merge(x, ktiles):
    """[ktiles*128, W] -> [128, ktiles*W] with k-tiles side by side."""
    w = x.shape[-1]
    return np.ascontiguousarray(
        x.reshape(ktiles, P, w).transpose(1, 0, 2).reshape(P, ktiles * w))


def _in_maps(inputs):
    f32 = lambda x: np.ascontiguousarray(np.asarray(x, dtype=np.float32))
    r = f32(inputs["r"])
    noise0 = f32(inputs["noise0"])
    noise = f32(inputs["noise"])
    W0 = f32(inputs["W0"]).astype(np.float64)
    b0 = f32(inputs["b0"]).astype(np.float64)
    Wh = f32(inputs["Wh"]).astype(np.float64)
    bh = f32(inputs["bh"]).astype(np.float64)
    Wo = f32(inputs["Wo"]).astype(np.float64)
    bo = f32(inputs["bo"]).astype(np.float64)
    Wt = f32(inputs["Wt"]).astype(np.float64)
    bt = f32(inputs["bt"]).astype(np.float64)

    # host-side weight folding (see module docstring)
    weff_dt = DT * (Wh @ Wh @ Wo)                        # [H, Z]
    const = (bh @ Wh + bh) @ Wo + bo                     # [Z]
    ts = np.arange(1, T + 1, dtype=np.float64) * DT
    temb = np.maximum(ts[:, None] * Wt[0][None, :] + bt[None, :], 0.0)
    c = temb @ W0 + b0                                   # [T, H]
    ctabb = np.ascontiguousarray(
        c.T.reshape(MH, P, T).transpose(1, 0, 2).reshape(P, MH * T)
    ).astype(np.float32)

    shared = {
        "identb": np.eye(P, dtype=np.float32).astype(BF16),
        "wrb": _ktile_merge(W0[Z:], KR).astype(BF16),
        "wzb": _ktile_merge(W0[:Z], KZ).astype(BF16),
        "weffb": _ktile_merge(weff_dt, KH).astype(BF16),
        "ctabb": ctabb,
    }

    rT = np.ascontiguousarray(r.T)                       # [R, B]
    z0T = np.ascontiguousarray(noise0.T)                 # [Z, B]
    # eps'[t] = sqrt_dt*eps + dt*const, feature-major, 2-step chunks
    epsp = (SQDT * noise.transpose(0, 2, 1)
            + DT * const.astype(np.float32)[None, :, None])  # [T, Z, B]
    maps = []
    for cix in range(NC):
        sl = slice(cix * BS, (cix + 1) * BS)
        m = dict(shared)
        m["rtb"] = _ktile_merge(
            np.ascontiguousarray(rT[:, sl]), KR).astype(BF16)
        m["z0bb"] = _ktile_merge(
            np.ascontiguousarray(z0T[:, sl]), KZ).astype(BF16)
        ec = np.ascontiguousarray(epsp[:, :, sl])        # [T, Z, BS]
        m["epsb"] = np.ascontiguousarray(
            ec.reshape(NCH, 2, KZ, P, BS).transpose(0, 3, 1, 2, 4)
            .reshape(NCH, P, 2 * ZB)).astype(BF16)
        maps.append(m)
    return maps, noise0, noise


def _run(inputs, **run_kwargs):
    from concourse.bass_utils import run_bass_kernel_spmd
    nc = _get_nc()
    maps, noise0, noise = _in_maps(inputs)
    res = run_bass_kernel_spmd(nc, maps, core_ids=list(range(NC)),
                               **run_kwargs)
    out = np.empty((3, T + 1, B, Z), np.float32)
    out[0, 0] = noise0
    out[1, 0] = 0.0
    out[2, 0] = 1.0
    out[2, 1:] = np.float32(SQDT)
    for cix in range(NC):
        sl = slice(cix * BS, (cix + 1) * BS)
        zs = res.results[cix]["zsb"].astype(np.float32)
        # [T, P, (mz, b)] -> [T, BS, Z]
        zs = (zs.reshape(T, P, KZ, BS).transpose(0, 3, 2, 1)
              .reshape(T, BS, Z))
        out[0, 1:, sl, :] = zs
    # mu = z - sqrt_dt*eps, exactly (reference: z = mu + sqrt_dt*eps)
    out[1, 1:] = out[0, 1:] - np.float32(SQDT) * noise
    return out, res


def kernel(**inputs) -> np.ndarray:
    out, _ = _run(inputs)
    return out
